# revision 16
# baseline (speedup 1.0000x reference)
"""Block-local sparse attention (LSG-style) on 8 TRN2 NeuronCores.

Sharding: the 32 (n, h) pairs are split 4-per-core (data/head parallel, no
collectives). Host-side numpy prep re-lays-out the inputs so the device
kernel needs no transposes, all bf16:

  - qt : Q^T per head [64, T]
  - lkt/skt/gkt: local/sparse/global K^T, token-padded with zeros
  - lv/sv/gv: V with a ones column appended (col 64), chunked [128, c, 65],
    and every row scaled by exp(mask): softmax(QK/8 + m) @ V is computed as
    sum_t exp(s_t) e^{m_t} [V_t, 1], then a divide by the accumulated last
    column. sv holds 4 phase-shifted copies so the 32-token-granular sparse
    windows always start at partition 0.

The device processes query-block PAIRS: 9 score matmuls per pair into a
3-bank PSUM region [128, 1536], one wide exp(S/8) on ACT, 12 PV matmuls
(N=65) accumulating [q, V|Z], and a reciprocal-normalize on DVE to bf16.

v2 scheduling (PE stream-bound at 0.836 ns/col; everything else hides
under it):
  - One semaphore increment per pair: the batch [scores(p+2), PV(p)] ends
    with a single then_inc(z) on the last PV matmul. Each mm-attached inc
    stalls the in-order PE queue ~115 ns waiting for stream completion, so
    v1's two incs/pair cost ~230 ns/pair.
  - Output stores ride the GpSimd queue (sync only issues loads), one
    bf16 store per pair (v1: two fp32 stores on sync, which head-of-line
    blocked behind input-load bursts, causing ~8 us stalls and a 10 us
    drain tail).
  - Loads are issued per-head with a pairs-0..5 prefix split so the PE
    starts after ~0.5 MB instead of 5 MB (v1 startup stall: ~21 us).
"""

from contextlib import ExitStack

import numpy as np

import concourse.bass as bass
import concourse.mybir as mybir
from concourse.bass_utils import run_bass_kernel_spmd

N, H, T, D = 2, 16, 4096, 64
B = 128          # query block
NB = T // B      # 32
G = 64           # global tokens
TSP = T // 4     # sparse tokens (1024)
NH = N * H       # 32
NCORES = 8
SL = NH // NCORES  # 4 heads per core
NP = SL * NB // 2  # 64 block-pairs per core
PPS = NB // 2      # 16 pairs per slot (slot == head)

LKT_W = T + 2 * B            # 4352 padded local tokens
SKT_W = TSP + 320            # 1344 padded sparse tokens
LV_C = LKT_W // 128          # 34 local V chunks
SV_C = 11                    # sparse V chunks per phase

F32 = mybir.dt.float32
BF16 = mybir.dt.bfloat16
I16 = mybir.dt.int16
# bf16 Schraudolph exp for the global chunk: bits16 = S*128/(8*ln2) + (16256-7)
SCHRAU_MUL = 128.0 / (8.0 * 0.6931471805599453)
SCHRAU_ADD = 16256.0 - 7.0
GE = "sem-ge"

# column layout of the per-pair score/prob tile [128, 1536] (3 PSUM banks;
# regions never cross a 512-col bank boundary)
# pp (probability tile) column layout, [128, 1536] bf16:
C_SP1A, C_SP1B = 0, 128
C_SP2A, C_SP2B = 256, 384
C_LOC1 = 512     # 256 wide: local chunk b+1, both blocks
C_LOC2 = 768     # 256 wide: local chunk b+2, both blocks
C_LOC0 = 1024    # 128: local chunk b, block A only
C_LOC3 = 1152    # 128: local chunk b+3, block B only
C_G = 1280       # 256 wide: global chunk, both blocks (true exp on ACT)
# Raw scores are split across two PSUM tensors so the engines that read
# them never share a PSUM bank-pair (concurrent ACT+DVE reads of nearby
# banks abort the NEFF): psA [128,1024] x2 (banks 0-3) holds locals+G,
# read by ACT's exp; psB [128,512] x2 (banks 4-5) holds the sparse slots,
# read by DVE's Schraudolph; pv x2 (banks 6-7) holds PV output + is read
# by DVE's copy. psA cols = pp cols - 512; psB cols = pp cols.

# load split: prefix covers pairs 0..PRE_P-1 (blocks 0..2*PRE_P-1) of head 0
PRE_P = 6
PRE_QT = 2 * PRE_P * B                 # qt cols
PRE_LKT = (2 * PRE_P + 3) * B          # lkt cols
PRE_SKT = 32 * (2 * PRE_P - 1) + 352   # skt cols
PRE_LV = (2 * PRE_P + 3) * 65          # lv cols
PRE_SVC = (32 * (2 * PRE_P - 1) + 224) // 128 + 1  # sv chunks per phase


def _build_bass():
    nc = bass.Bass("TRN2", num_devices=NCORES, debug=False)

    qt = nc.dram_tensor("qt", [SL, 128, T], BF16, kind="ExternalInput")
    lkt = nc.dram_tensor("lkt", [SL, 128, LKT_W], BF16, kind="ExternalInput")
    skt = nc.dram_tensor("skt", [SL, 128, SKT_W], BF16, kind="ExternalInput")
    gkt = nc.dram_tensor("gkt", [SL, 128, 128], BF16, kind="ExternalInput")
    lv = nc.dram_tensor("lv", [SL, 128, LV_C * 65], BF16, kind="ExternalInput")
    sv = nc.dram_tensor("sv", [SL, 128, 4 * SV_C * 65], BF16, kind="ExternalInput")
    gv = nc.dram_tensor("gv", [SL, 128, 65], BF16, kind="ExternalInput")
    o = nc.dram_tensor("o", [SL, T, 65], BF16, kind="ExternalOutput")

    EXP = mybir.ActivationFunctionType.Exp

    with ExitStack() as es:
        ec = es.enter_context
        # double-buffered inputs (head parity)
        qt_t = [ec(nc.sbuf_tensor(f"qt_t{i}", [128, T], BF16)) for i in range(2)]
        lkt_t = [ec(nc.sbuf_tensor(f"lkt_t{i}", [128, LKT_W], BF16)) for i in range(2)]
        skt_t = [ec(nc.sbuf_tensor(f"skt_t{i}", [128, SKT_W], BF16)) for i in range(2)]
        gkt_t = [ec(nc.sbuf_tensor(f"gkt_t{i}", [128, 128], BF16)) for i in range(2)]
        lv_t = [ec(nc.sbuf_tensor(f"lv_t{i}", [128, LV_C * 65], BF16)) for i in range(2)]
        sv_t = [ec(nc.sbuf_tensor(f"sv_t{i}", [128, 4 * SV_C * 65], BF16)) for i in range(2)]
        gv_t = [ec(nc.sbuf_tensor(f"gv_t{i}", [128, 65], BF16)) for i in range(2)]
        # double-buffered per-pair working set (pair parity)
        psA = [ec(nc.psum_tensor(f"psA{i}", [128, 1024], F32)) for i in range(2)]  # banks 0-3
        psB = [ec(nc.psum_tensor(f"psB{i}", [128, 512], F32)) for i in range(2)]   # banks 4-5
        pv = [ec(nc.psum_tensor(f"pv{i}", [128, 512], F32)) for i in range(2)]     # banks 6-7
        pp = [ec(nc.sbuf_tensor(f"pp{i}", [128, 1536], BF16)) for i in range(2)]
        ob = [ec(nc.sbuf_tensor(f"ob{i}", [128, 2, 65], BF16)) for i in range(4)]

        diK = ec(nc.semaphore("diK"))  # K-side loads (+16 each)
        diV = ec(nc.semaphore("diV"))  # V-side loads (+16 each)
        z = ec(nc.semaphore("z"))      # +1 per emitted score-batch / PE batch
        act = ec(nc.semaphore("act"))  # +1 per pair: ACT exp done
        dact = ec(nc.semaphore("dact"))  # +1 per pair: DVE schrau exp done
        dve = ec(nc.semaphore("dve"))  # +1 per pair: normalize done
        st = ec(nc.semaphore("st"))    # +16 per pair: store done
        block = ec(nc.Block(no_gpsimd_drain=True))

        # cumulative DMA counts per head: head 0 K = 7 (4 prefix + 3 rest),
        # V = 5 (3 prefix + 2 rest); heads 1..3: K = 4, V = 3.
        K_CUM = [4, 7, 11, 15, 19]   # [h0 prefix, h0 full, h1, h2, h3]
        V_CUM = [3, 5, 8, 11, 14]

        @block.sync
        def _(sync):
            def load_head_k(s, u, prefix_only=False, rest_only=False):
                if s == 0:
                    if not rest_only:
                        for dst, src in (
                            (qt_t[u][:, 0:PRE_QT], qt[0, :, 0:PRE_QT]),
                            (lkt_t[u][:, 0:PRE_LKT], lkt[0, :, 0:PRE_LKT]),
                            (skt_t[u][:, 0:PRE_SKT], skt[0, :, 0:PRE_SKT]),
                            (gkt_t[u][:], gkt[0]),
                        ):
                            sync.dma_start(dst, src).then_inc(diK, 16)
                    if not prefix_only:
                        for dst, src in (
                            (qt_t[u][:, PRE_QT:T], qt[0, :, PRE_QT:T]),
                            (lkt_t[u][:, PRE_LKT:LKT_W], lkt[0, :, PRE_LKT:LKT_W]),
                            (skt_t[u][:, PRE_SKT:SKT_W], skt[0, :, PRE_SKT:SKT_W]),
                        ):
                            sync.dma_start(dst, src).then_inc(diK, 16)
                else:
                    for dst, src in (
                        (qt_t[u][:], qt[s]),
                        (lkt_t[u][:], lkt[s]),
                        (skt_t[u][:], skt[s]),
                        (gkt_t[u][:], gkt[s]),
                    ):
                        sync.dma_start(dst, src).then_inc(diK, 16)

            def load_head_v(s, u, prefix_only=False, rest_only=False):
                if s == 0:
                    sv3 = sv_t[u][:].rearrange("p (ph c) -> p ph c", ph=4)
                    sv3s = sv[0].rearrange("p (ph c) -> p ph c", ph=4)
                    pc = PRE_SVC * 65
                    if not rest_only:
                        for dst, src in (
                            (lv_t[u][:, 0:PRE_LV], lv[0, :, 0:PRE_LV]),
                            (sv3[:, :, 0:pc], sv3s[:, :, 0:pc]),
                            (gv_t[u][:], gv[0]),
                        ):
                            sync.dma_start(dst, src).then_inc(diV, 16)
                    if not prefix_only:
                        for dst, src in (
                            (lv_t[u][:, PRE_LV:], lv[0, :, PRE_LV:]),
                            (sv3[:, :, pc:], sv3s[:, :, pc:]),
                        ):
                            sync.dma_start(dst, src).then_inc(diV, 16)
                else:
                    for dst, src in (
                        (lv_t[u][:], lv[s]),
                        (sv_t[u][:], sv[s]),
                        (gv_t[u][:], gv[s]),
                    ):
                        sync.dma_start(dst, src).then_inc(diV, 16)

            load_head_k(0, 0, prefix_only=True)
            load_head_v(0, 0, prefix_only=True)
            load_head_k(0, 0, rest_only=True)
            load_head_v(0, 0, rest_only=True)
            load_head_k(1, 1)
            load_head_v(1, 1)
            # head 2 K may overwrite qt_t[0] once scores(15) done (batch 13,
            # z = 16 counting the preamble inc); V once PV(15) done (z = 18).
            sync.dma_start(
                qt_t[0][:], qt[2]
            ).wait_op(z, PPS, GE).then_inc(diK, 16)
            for dst, src in ((lkt_t[0][:], lkt[2]), (skt_t[0][:], skt[2]),
                             (gkt_t[0][:], gkt[2])):
                sync.dma_start(dst, src).then_inc(diK, 16)
            sync.dma_start(
                lv_t[0][:], lv[2]
            ).wait_op(z, PPS + 2, GE).then_inc(diV, 16)
            for dst, src in ((sv_t[0][:], sv[2]), (gv_t[0][:], gv[2])):
                sync.dma_start(dst, src).then_inc(diV, 16)
            # head 3
            sync.dma_start(
                qt_t[1][:], qt[3]
            ).wait_op(z, 2 * PPS, GE).then_inc(diK, 16)
            for dst, src in ((lkt_t[1][:], lkt[3]), (skt_t[1][:], skt[3]),
                             (gkt_t[1][:], gkt[3])):
                sync.dma_start(dst, src).then_inc(diK, 16)
            sync.dma_start(
                lv_t[1][:], lv[3]
            ).wait_op(z, 2 * PPS + 2, GE).then_inc(diV, 16)
            for dst, src in ((sv_t[1][:], sv[3]), (gv_t[1][:], gv[3])):
                sync.dma_start(dst, src).then_inc(diV, 16)
            sync.wait_ge(st, 16 * NP)

        def emit_scores(p, gate_act=None, inc_z=False):
            s, hb = divmod(p, PPS)
            b = 2 * hb
            u = p % 2
            su = s % 2
            qA = qt_t[su][:, b * B : (b + 1) * B]
            qB = qt_t[su][:, (b + 1) * B : (b + 2) * B]
            qAB = qt_t[su][:, b * B : (b + 2) * B]
            w1a, w2a = 32 * b, 32 * b + 224
            w1b, w2b = w1a + 32, w2a + 32
            # ACT-region (psA) matmuls first: the act-gate rides mm #1;
            # the sparse (psB, DVE-read) matmuls come after the tucked
            # dact-wait so they never overwrite what the DVE still reads.
            mms = (
                (C_LOC1, 256, lkt_t[su][:, (b + 1) * B : (b + 2) * B], qAB),
                (C_LOC0, 128, lkt_t[su][:, b * B : (b + 1) * B], qA),
                (C_LOC2, 256, lkt_t[su][:, (b + 2) * B : (b + 3) * B], qAB),
                (C_LOC3, 128, lkt_t[su][:, (b + 3) * B : (b + 4) * B], qB),
                (C_G, 256, gkt_t[su][:, :], qAB),
                (C_SP1A, 128, skt_t[su][:, w1a : w1a + 128], qA),
                (C_SP1B, 128, skt_t[su][:, w1b : w1b + 128], qB),
                (C_SP2A, 128, skt_t[su][:, w2a : w2a + 128], qA),
                (C_SP2B, 128, skt_t[su][:, w2b : w2b + 128], qB),
            )
            for kk, (col, w, lhsT, rhs) in enumerate(mms):
                out_ap = (psB[u][:, col : col + w] if col < 512
                          else psA[u][:, col - 512 : col - 512 + w])
                mm = nc.tensor.matmul(
                    out_ap,
                    lhsT, rhs,
                    start=True, stop=True,
                )
                if kk == 0 and gate_act is not None:
                    mm.wait_op(act, gate_act, GE)  # psS[u] free (exp done)
                if kk == 0 and gate_act is not None and p >= 4:
                    # pv free + schrau(p-2) done, checked here (pre-satisfied)
                    # so the queue checks hide under this matmul's stream.
                    nc.tensor.wait_ge(dve, p - 3)
                    nc.tensor.wait_ge(dact, p - 1)
                if kk == len(mms) - 1 and inc_z:
                    mm.then_inc(z, 1)

        def emit_pv(p, gate_act=None):
            s, hb = divmod(p, PPS)
            b = 2 * hb
            u = p % 2
            su = s % 2
            if gate_act is not None and p >= 2:
                # no scores batch ahead of this PV to carry the act gate;
                # the dve wait moves to a standalone so the first mm can
                # carry the act wait (one sem wait per instruction).
                nc.tensor.wait_ge(dve, p - 1)
                nc.tensor.wait_ge(dact, p + 1)
            for blk in range(2):
                bb = b + blk
                w1, w2 = 32 * bb, 32 * bb + 224
                c1, r1 = divmod(w1, 128)
                c2, r2 = divmod(w2, 128)
                sp1c = ((r1 // 32) * SV_C + c1) * 65
                sp2c = ((r2 // 32) * SV_C + c2) * 65
                if blk == 0:
                    lhs = (C_SP1A, C_SP2A, C_G, C_LOC0, C_LOC1, C_LOC2)
                else:
                    lhs = (C_SP1B, C_SP2B, C_G + 128, C_LOC1 + 128,
                           C_LOC2 + 128, C_LOC3)
                # (layout-independent: cols come from the C_* constants)
                rhss = (
                    sv_t[su][:, sp1c : sp1c + 65],
                    sv_t[su][:, sp2c : sp2c + 65],
                    gv_t[su][:],
                    lv_t[su][:, bb * 65 : bb * 65 + 65],
                    lv_t[su][:, (bb + 1) * 65 : (bb + 1) * 65 + 65],
                    lv_t[su][:, (bb + 2) * 65 : (bb + 2) * 65 + 65],
                )
                out = pv[u][:, blk * 128 : blk * 128 + 65]
                for j in range(6):
                    mm = nc.tensor.matmul(
                        out, pp[u][:, lhs[j] : lhs[j] + 128], rhss[j],
                        start=(j == 0), stop=(j == 5),
                    )
                    if blk == 0 and j == 0 and gate_act is not None:
                        mm.wait_op(act, gate_act, GE)  # pp[u] ready
                    if blk == 1 and j == 5:
                        mm.then_inc(z, 1)

        @block.tensor
        def _(tensor):
            tensor.wait_ge(diK, 16 * K_CUM[0])
            emit_scores(0)
            emit_scores(1, inc_z=True)  # z=1: scores(0,1) complete
            for p in range(NP):
                s, hb = divmod(p, PPS)
                if p + 2 < NP:
                    # scores(p+2) may need K beyond what's loaded-gated so far
                    s2, hb2 = divmod(p + 2, PPS)
                    if s2 == 0 and hb2 == PRE_P:
                        tensor.wait_ge(diK, 16 * K_CUM[1])
                    elif hb2 == 0 and s2 > 0:
                        tensor.wait_ge(diK, 16 * K_CUM[s2 + 1])
                    emit_scores(p + 2, gate_act=p + 1)
                if s == 0 and hb == 0:
                    tensor.wait_ge(diV, 16 * V_CUM[0])
                elif s == 0 and hb == PRE_P:
                    tensor.wait_ge(diV, 16 * V_CUM[1])
                elif hb == 0:
                    tensor.wait_ge(diV, 16 * V_CUM[s + 1])
                emit_pv(p, gate_act=p + 1 if p + 2 >= NP else None)

        @block.scalar
        def _(scalar):
            for p in range(NP):
                u = p % 2
                nc.scalar.activation(
                    pp[u][:, 512:1536], psA[u][:, 0:1024], EXP, scale=0.125
                ).wait_op(z, max(1, p), GE).then_inc(act, 1)

        def emit_schrau(p, gate_z=None):
            u = p % 2
            ts = nc.vector.tensor_scalar(
                pp[u][:, 0:512].bitcast(I16),
                psB[u][:, 0:512],
                SCHRAU_MUL, SCHRAU_ADD,
                op0=mybir.AluOpType.mult, op1=mybir.AluOpType.add,
            )
            if gate_z is not None:
                ts.wait_op(z, gate_z, GE)
            ts.then_inc(dact, 1)

        @block.vector
        def _(vector):
            emit_schrau(0, gate_z=1)
            emit_schrau(1)
            for p in range(NP):
                u = p % 2
                u4 = p % 4
                if p >= 4:
                    vector.wait_ge(st, 16 * (p - 3))  # ob[u4] free
                pv3 = pv[u][:].rearrange("q (a c) -> q a c", a=4)
                nc.vector.tensor_copy(
                    ob[u4][:], pv3[:, 0:2, 0:65]
                ).wait_op(z, p + 2, GE).then_inc(dve, 1)
                if p + 2 < NP:
                    # schrau(p+2): psS[p%2] bank 0 was written by scores(p+2),
                    # complete per the z>=p+2 wait above.
                    emit_schrau(p + 2)

        @block.gpsimd
        def _(gpsimd):
            for p in range(NP):
                s, hb = divmod(p, PPS)
                b = 2 * hb
                dst = o[s, b * B : (b + 2) * B, :].rearrange(
                    "(blk q) d -> q blk d", blk=2
                )
                gpsimd.dma_start(dst, ob[p % 4][:]).wait_op(
                    dve, p + 1, GE
                ).then_inc(st, 16)

    return nc


def _prepare(inputs):
    import ml_dtypes

    bf = ml_dtypes.bfloat16
    f = np.float32
    q = np.asarray(inputs["query_layer"], f).reshape(NH, T, D)
    k = np.asarray(inputs["key_layer"], f).reshape(NH, T, D)
    v = np.asarray(inputs["value_layer"], f).reshape(NH, T, D)
    sk = np.asarray(inputs["sparse_key"], f).reshape(NH, TSP, D)
    svv = np.asarray(inputs["sparse_value"], f).reshape(NH, TSP, D)
    gk = np.asarray(inputs["global_key"], f).reshape(NH, G, D)
    gvv = np.asarray(inputs["global_value"], f).reshape(NH, G, D)
    am = np.repeat(np.asarray(inputs["attention_mask"], f)[:, 0, 0, :], H, 0)
    sm = np.repeat(np.asarray(inputs["sparse_mask"], f)[:, 0, 0, :], H, 0)
    gm = np.repeat(np.asarray(inputs["global_mask"], f)[:, 0, 0, :], H, 0)

    qt = np.zeros((NH, 128, T), f)
    qt[:, :64] = q.transpose(0, 2, 1)
    qt = qt.astype(bf)

    lkt = np.zeros((NH, 128, LKT_W), f)
    lkt[:, :64, B : B + T] = k.transpose(0, 2, 1)
    lkt = lkt.astype(bf)

    skt = np.zeros((NH, 128, SKT_W), f)
    skt[:, :64, 160 : 160 + TSP] = sk.transpose(0, 2, 1)
    skt = skt.astype(bf)

    gkt = np.zeros((NH, 128, 128), f)
    gkt[:, :64, :G] = gk.transpose(0, 2, 1)
    gkt = gkt.astype(bf)

    # V_aug rows scaled by exp(mask); pad rows are all-zero
    em_l = np.zeros((NH, LKT_W), f)
    em_l[:, B : B + T] = np.exp(am)
    lvp = np.zeros((NH, LKT_W, 65), f)
    lvp[:, B : B + T, :64] = v
    lvp[:, :, 64] = 1.0
    lvp *= em_l[:, :, None]
    lvp = np.ascontiguousarray(
        lvp.reshape(NH, LV_C, 128, 65).transpose(0, 2, 1, 3)
    ).reshape(NH, 128, LV_C * 65).astype(bf)

    SVP_W = 96 + SV_C * 128
    em_s = np.zeros((NH, SVP_W), f)
    em_s[:, 160 : 160 + TSP] = np.exp(sm)
    sv_pad = np.zeros((NH, SVP_W, 65), f)
    sv_pad[:, 160 : 160 + TSP, :64] = svv
    sv_pad[:, :, 64] = 1.0
    sv_pad *= em_s[:, :, None]
    svp = np.empty((NH, 4, 128, SV_C, 65), f)
    for p in range(4):
        svp[:, p] = (
            sv_pad[:, 32 * p : 32 * p + SV_C * 128]
            .reshape(NH, SV_C, 128, 65)
            .transpose(0, 2, 1, 3)
        )
    svp = np.ascontiguousarray(svp.transpose(0, 2, 1, 3, 4)).reshape(
        NH, 128, 4 * SV_C * 65
    ).astype(bf)

    gvp = np.zeros((NH, 128, 65), f)
    gvp[:, :G, :64] = gvv
    gvp[:, :G, 64] = 1.0
    gvp[:, :G] *= np.exp(gm)[:, :, None]
    gvp = gvp.astype(bf)

    return [
        {
            "qt": qt[c * SL : (c + 1) * SL],
            "lkt": lkt[c * SL : (c + 1) * SL],
            "skt": skt[c * SL : (c + 1) * SL],
            "gkt": gkt[c * SL : (c + 1) * SL],
            "lv": lvp[c * SL : (c + 1) * SL],
            "sv": svp[c * SL : (c + 1) * SL],
            "gv": gvp[c * SL : (c + 1) * SL],
        }
        for c in range(NCORES)
    ]


_NC_CACHE = {}
LAST_RESULTS = None


def kernel(**inputs):
    global LAST_RESULTS
    if "nc" not in _NC_CACHE:
        _NC_CACHE["nc"] = _build_bass()
    nc = _NC_CACHE["nc"]
    in_maps = _prepare(inputs)
    res = run_bass_kernel_spmd(nc, in_maps, core_ids=list(range(NCORES)))
    LAST_RESULTS = res
    out = np.empty((NH, T, D), np.float32)
    for c in range(NCORES):
        raw = res.results[c]["o"].astype(np.float32)  # [SL, T, 65]
        out[c * SL : (c + 1) * SL] = raw[:, :, :64] / raw[:, :, 64:65]
    return out.reshape(N, H, T, D)


# revision 24
# speedup vs baseline: 1.0572x; 1.0572x over previous
"""Block-local sparse attention (LSG-style) on 8 TRN2 NeuronCores.

Sharding: the 32 (n, h) pairs are split 4-per-core (data/head parallel, no
collectives). Host-side numpy prep re-lays-out the inputs so the device
kernel needs no transposes, all bf16:

  - qt : Q^T per head [64, T]
  - lkt/skt/gkt: local/sparse/global K^T, token-padded with zeros
  - lv/sv/gv: V with a ones column appended (col 64), chunked [128, c, 65],
    and every row scaled by exp(mask): softmax(QK/8 + m) @ V is computed as
    sum_t exp(s_t) e^{m_t} [V_t, 1], then a divide by the accumulated last
    column. sv holds 4 phase-shifted copies so the 32-token-granular sparse
    windows always start at partition 0.

The device processes query-block PAIRS: 9 score matmuls per pair into a
3-bank PSUM region [128, 1536], one wide exp(S/8) on ACT, 12 PV matmuls
(N=65) accumulating [q, V|Z], and a reciprocal-normalize on DVE to bf16.

v2 scheduling (PE stream-bound at 0.836 ns/col; everything else hides
under it):
  - One semaphore increment per pair: the batch [scores(p+2), PV(p)] ends
    with a single then_inc(z) on the last PV matmul. Each mm-attached inc
    stalls the in-order PE queue ~115 ns waiting for stream completion, so
    v1's two incs/pair cost ~230 ns/pair.
  - Output stores ride the GpSimd queue (sync only issues loads), one
    bf16 store per pair (v1: two fp32 stores on sync, which head-of-line
    blocked behind input-load bursts, causing ~8 us stalls and a 10 us
    drain tail).
  - Loads are issued per-head with a pairs-0..5 prefix split so the PE
    starts after ~0.5 MB instead of 5 MB (v1 startup stall: ~21 us).
"""

from contextlib import ExitStack

import numpy as np

import concourse.bass as bass
import concourse.mybir as mybir
from concourse.bass_utils import run_bass_kernel_spmd

N, H, T, D = 2, 16, 4096, 64
B = 128          # query block
NB = T // B      # 32
G = 64           # global tokens
TSP = T // 4     # sparse tokens (1024)
NH = N * H       # 32
NCORES = 8
SL = NH // NCORES  # 4 heads per core
NP = SL * NB // 2  # 64 block-pairs per core
PPS = NB // 2      # 16 pairs per slot (slot == head)

LKT_W = T + 2 * B            # 4352 padded local tokens
SKT_W = TSP + 320            # 1344 padded sparse tokens
LV_C = LKT_W // 128          # 34 local V chunks
SV_C = 11                    # sparse V chunks per phase

F32 = mybir.dt.float32
BF16 = mybir.dt.bfloat16
I16 = mybir.dt.int16
# bf16 Schraudolph exp for the global chunk: bits16 = S*128/(8*ln2) + (16256-7)
SCHRAU_MUL = 128.0 / (8.0 * 0.6931471805599453)
SCHRAU_ADD = 16256.0 - 7.0
GE = "sem-ge"

# column layout of the per-pair score/prob tile [128, 1536] (3 PSUM banks;
# regions never cross a 512-col bank boundary)
# pp (probability tile) column layout, [128, 1536] bf16:
C_SP1A, C_SP1B = 0, 128
C_SP2A, C_SP2B = 256, 384
C_LOC1 = 512     # 256 wide: local chunk b+1, both blocks
C_LOC2 = 768     # 256 wide: local chunk b+2, both blocks
C_LOC0 = 1024    # 128: local chunk b, block A only
C_LOC3 = 1152    # 128: local chunk b+3, block B only
C_G = 1280       # 256 wide: global chunk, both blocks (true exp on ACT)
# Raw scores are split across two PSUM tensors so the engines that read
# them never share a PSUM bank-pair (concurrent ACT+DVE reads of nearby
# banks abort the NEFF): psA [128,1024] x2 (banks 0-3) holds locals+G,
# read by ACT's exp; psB [128,512] x2 (banks 4-5) holds the sparse slots,
# read by DVE's Schraudolph; pv x2 (banks 6-7) holds PV output + is read
# by DVE's copy. psA cols = pp cols - 512; psB cols = pp cols.

# load split: prefix covers pairs 0..PRE_P-1 (blocks 0..2*PRE_P-1) of head 0
PRE_P = 6
PRE_QT = 2 * PRE_P * B                 # qt cols
PRE_LKT = (2 * PRE_P + 3) * B          # lkt cols
PRE_SKT = 32 * (2 * PRE_P - 1) + 352   # skt cols
PRE_LV = (2 * PRE_P + 3) * 65          # lv cols
PRE_SVC = (32 * (2 * PRE_P - 1) + 224) // 128 + 1  # sv chunks per phase


def _build_bass():
    nc = bass.Bass("TRN2", num_devices=NCORES, debug=False)

    qt = nc.dram_tensor("qt", [SL, 128, T], BF16, kind="ExternalInput")
    lkt = nc.dram_tensor("lkt", [SL, 64, LKT_W], BF16, kind="ExternalInput")
    skt = nc.dram_tensor("skt", [SL, 64, SKT_W], BF16, kind="ExternalInput")
    gkt = nc.dram_tensor("gkt", [SL, 64, 128], BF16, kind="ExternalInput")
    lv = nc.dram_tensor("lv", [SL, 128, LV_C * 65], BF16, kind="ExternalInput")
    sv = nc.dram_tensor("sv", [SL, 128, 4 * SV_C * 65], BF16, kind="ExternalInput")
    gv = nc.dram_tensor("gv", [SL, 128, 65], BF16, kind="ExternalInput")
    kz = nc.dram_tensor("kz", [64, LKT_W], BF16, kind="ExternalInput")  # zeros
    o = nc.dram_tensor("o", [SL, T, 65], BF16, kind="ExternalOutput")

    EXP = mybir.ActivationFunctionType.Exp

    with ExitStack() as es:
        ec = es.enter_context
        # double-buffered inputs (head parity)
        qt_t = [ec(nc.sbuf_tensor(f"qt_t{i}", [128, T], BF16)) for i in range(2)]
        lkt_t = [ec(nc.sbuf_tensor(f"lkt_t{i}", [128, LKT_W], BF16)) for i in range(2)]
        skt_t = [ec(nc.sbuf_tensor(f"skt_t{i}", [128, SKT_W], BF16)) for i in range(2)]
        gkt_t = [ec(nc.sbuf_tensor(f"gkt_t{i}", [128, 128], BF16)) for i in range(2)]
        lv_t = [ec(nc.sbuf_tensor(f"lv_t{i}", [128, LV_C * 65], BF16)) for i in range(2)]
        sv_t = [ec(nc.sbuf_tensor(f"sv_t{i}", [128, 4 * SV_C * 65], BF16)) for i in range(2)]
        gv_t = [ec(nc.sbuf_tensor(f"gv_t{i}", [128, 65], BF16)) for i in range(2)]
        # double-buffered per-pair working set (pair parity)
        psA = [ec(nc.psum_tensor(f"psA{i}", [128, 1024], F32)) for i in range(2)]  # banks 0-3
        psB = [ec(nc.psum_tensor(f"psB{i}", [128, 512], F32)) for i in range(2)]   # banks 4-5
        pv = [ec(nc.psum_tensor(f"pv{i}", [128, 512], F32)) for i in range(2)]     # banks 6-7
        pp = [ec(nc.sbuf_tensor(f"pp{i}", [128, 1536], BF16)) for i in range(4)]
        ob = [ec(nc.sbuf_tensor(f"ob{i}", [128, 2, 65], BF16)) for i in range(4)]

        # one semaphore per load tier: DMA completions are out-of-order
        # across engines, so a shared counter can hit its threshold with a
        # later DMA substituting for an unfinished earlier one.
        diK = [ec(nc.semaphore(f"diK{i}")) for i in range(6)]
        diV = [ec(nc.semaphore(f"diV{i}")) for i in range(6)]
        z = ec(nc.semaphore("z"))      # tail marker: +1 per PV of the last 2 batches
        zs = ec(nc.semaphore("zs"))    # +1 per completed scores(j): zs = j+1
        mz = [ec(nc.semaphore(f"mz{i}")) for i in range(2)]
        act = ec(nc.semaphore("act"))  # +1 per pair: ACT exp done
        dact = ec(nc.semaphore("dact"))  # +1 per pair: DVE schrau exp done
        dve = ec(nc.semaphore("dve"))  # +1 per pair: normalize done
        st = ec(nc.semaphore("st"))    # +16 per pair: store done
        block = ec(nc.Block(no_gpsimd_drain=True))

        # DMAs per tier (x16 per DMA):
        # K: Kpre0(4: pairs 0-1) Kpre1(3: ..pair 5) Krest(3) h1(4) h2(4) h3(4)
        # V: Vpre0(3: pairs 0-1) Vpre1(1: ..pair 5) Vrest(2) h1(3) h2(3) h3(3)
        K_TIER = [4, 3, 3, 4, 4, 4]
        V_TIER = [3, 1, 2, 3, 3, 3]

        @block.sync
        def _(sync):
            KP0_QT, KP1_QT = 512, 1536
            KP0_LKT, KP1_LKT = 896, PRE_LKT
            KP0_SKT, KP1_SKT = 448, PRE_SKT

            def load_head_k(s, u, sem):
                for dst, srcp in (
                    (qt_t[u][:], qt[s]),
                    (lkt_t[u][0:64, :], lkt[s]),
                    (skt_t[u][0:64, :], skt[s]),
                    (gkt_t[u][0:64, :], gkt[s]),
                ):
                    dma = sync.dma_start(dst, srcp)
                    dma.then_inc(sem, 16)
                    yield dma

            def load_head_v(s, u, sem):
                for dst, srcp in (
                    (lv_t[u][:], lv[s]),
                    (sv_t[u][:], sv[s]),
                    (gv_t[u][:], gv[s]),
                ):
                    dma = sync.dma_start(dst, srcp)
                    dma.then_inc(sem, 16)
                    yield dma

            # head 0, finely tiered
            for dst, srcp in (
                (qt_t[0][:, 0:KP0_QT], qt[0, :, 0:KP0_QT]),
                (lkt_t[0][0:64, 0:KP0_LKT], lkt[0, :, 0:KP0_LKT]),
                (skt_t[0][0:64, 0:KP0_SKT], skt[0, :, 0:KP0_SKT]),
                (gkt_t[0][0:64, :], gkt[0]),
            ):
                sync.dma_start(dst, srcp).then_inc(diK[0], 16)
            for dst, srcp in (
                (qt_t[0][:, KP0_QT:KP1_QT], qt[0, :, KP0_QT:KP1_QT]),
                (lkt_t[0][0:64, KP0_LKT:KP1_LKT], lkt[0, :, KP0_LKT:KP1_LKT]),
                (skt_t[0][0:64, KP0_SKT:KP1_SKT], skt[0, :, KP0_SKT:KP1_SKT]),
            ):
                sync.dma_start(dst, srcp).then_inc(diK[1], 16)
            sv3 = sv_t[0][:].rearrange("p (ph c) -> p ph c", ph=4)
            sv3s = sv[0].rearrange("p (ph c) -> p ph c", ph=4)
            for dst, srcp in (
                (lv_t[0][:, 0:PRE_LV], lv[0, :, 0:PRE_LV]),
                (sv3[:, :, 0 : 3 * 65], sv3s[:, :, 0 : 3 * 65]),
                (gv_t[0][:], gv[0]),
            ):
                sync.dma_start(dst, srcp).then_inc(diV[0], 16)
            sync.dma_start(
                sv3[:, :, 3 * 65 : 5 * 65], sv3s[:, :, 3 * 65 : 5 * 65]
            ).then_inc(diV[1], 16)
            for dst, srcp in (
                (qt_t[0][:, KP1_QT:T], qt[0, :, KP1_QT:T]),
                (lkt_t[0][0:64, KP1_LKT:LKT_W], lkt[0, :, KP1_LKT:LKT_W]),
                (skt_t[0][0:64, KP1_SKT:SKT_W], skt[0, :, KP1_SKT:SKT_W]),
            ):
                sync.dma_start(dst, srcp).then_inc(diK[2], 16)
            for dst, srcp in (
                (lv_t[0][:, PRE_LV:], lv[0, :, PRE_LV:]),
                (sv3[:, :, 5 * 65 :], sv3s[:, :, 5 * 65 :]),
            ):
                sync.dma_start(dst, srcp).then_inc(diV[2], 16)
            # head 1 unconditionally
            list(load_head_k(1, 1, diK[3]))
            list(load_head_v(1, 1, diV[3]))
            # heads 2 and 3 gated on scores/PV completion (zs markers)
            for first, dma in enumerate(load_head_k(2, 0, diK[4])):
                if first == 0:
                    dma.wait_op(zs, PPS, GE)
            for first, dma in enumerate(load_head_v(2, 0, diV[4])):
                if first == 0:
                    dma.wait_op(zs, PPS + 3, GE)
            for first, dma in enumerate(load_head_k(3, 1, diK[5])):
                if first == 0:
                    dma.wait_op(zs, 2 * PPS, GE)
            for first, dma in enumerate(load_head_v(3, 1, diV[5])):
                if first == 0:
                    dma.wait_op(zs, 2 * PPS + 3, GE)
            sync.wait_ge(st, 16 * NP)

        def emit_scores(p, gate_act=None):
            s, hb = divmod(p, PPS)
            b = 2 * hb
            u = p % 2
            su = s % 2
            qA = qt_t[su][:, b * B : (b + 1) * B]
            qB = qt_t[su][:, (b + 1) * B : (b + 2) * B]
            qAB = qt_t[su][:, b * B : (b + 2) * B]
            w1a, w2a = 32 * b, 32 * b + 224
            w1b, w2b = w1a + 32, w2a + 32
            # ACT-region (psA) matmuls first: the act-gate rides mm #1;
            # the sparse (psB, DVE-read) matmuls come after the tucked
            # dact-wait so they never overwrite what the DVE still reads.
            mms = (
                (C_LOC1, 256, lkt_t[su][:, (b + 1) * B : (b + 2) * B], qAB),
                (C_LOC0, 128, lkt_t[su][:, b * B : (b + 1) * B], qA),
                (C_LOC2, 256, lkt_t[su][:, (b + 2) * B : (b + 3) * B], qAB),
                (C_LOC3, 128, lkt_t[su][:, (b + 3) * B : (b + 4) * B], qB),
                (C_G, 256, gkt_t[su][:, :], qAB),
                (C_SP1A, 128, skt_t[su][:, w1a : w1a + 128], qA),
                (C_SP1B, 128, skt_t[su][:, w1b : w1b + 128], qB),
                (C_SP2A, 128, skt_t[su][:, w2a : w2a + 128], qA),
                (C_SP2B, 128, skt_t[su][:, w2b : w2b + 128], qB),
            )
            for kk, (col, w, lhsT, rhs) in enumerate(mms):
                out_ap = (psB[u][:, col : col + w] if col < 512
                          else psA[u][:, col - 512 : col - 512 + w])
                mm = nc.tensor.matmul(
                    out_ap,
                    lhsT, rhs,
                    start=True, stop=True,
                )
                if kk == 0 and gate_act is not None:
                    mm.wait_op(act, gate_act, GE)  # psS[u] free (exp done)
                if kk == 0 and gate_act is not None:
                    # pv free + schrau(p-2) done, checked here (pre-satisfied)
                    # so the queue checks hide under this matmul's stream.
                    if p >= 4:
                        nc.tensor.wait_ge(dve, p - 3)
                    if p >= 2:
                        nc.tensor.wait_ge(dact, p - 1)
                if kk == len(mms) - 1:
                    mm.then_inc(zs, 1)

        def emit_pv(p, gate_act=None):
            s, hb = divmod(p, PPS)
            b = 2 * hb
            u = p % 2
            su = s % 2
            if gate_act is not None and p >= 2:
                # no scores batch ahead of this PV to carry the act gate;
                # the dve wait moves to a standalone so the first mm can
                # carry the act wait (one sem wait per instruction).
                nc.tensor.wait_ge(dve, p - 1)
                nc.tensor.wait_ge(dact, p + 1)
            for blk in range(2):
                bb = b + blk
                w1, w2 = 32 * bb, 32 * bb + 224
                c1, r1 = divmod(w1, 128)
                c2, r2 = divmod(w2, 128)
                sp1c = ((r1 // 32) * SV_C + c1) * 65
                sp2c = ((r2 // 32) * SV_C + c2) * 65
                if blk == 0:
                    lhs = (C_SP1A, C_SP2A, C_G, C_LOC0, C_LOC1, C_LOC2)
                else:
                    lhs = (C_SP1B, C_SP2B, C_G + 128, C_LOC1 + 128,
                           C_LOC2 + 128, C_LOC3)
                # (layout-independent: cols come from the C_* constants)
                rhss = (
                    sv_t[su][:, sp1c : sp1c + 65],
                    sv_t[su][:, sp2c : sp2c + 65],
                    gv_t[su][:],
                    lv_t[su][:, bb * 65 : bb * 65 + 65],
                    lv_t[su][:, (bb + 1) * 65 : (bb + 1) * 65 + 65],
                    lv_t[su][:, (bb + 2) * 65 : (bb + 2) * 65 + 65],
                )
                out = pv[u][:, blk * 128 : blk * 128 + 65]
                for j in range(6):
                    mm = nc.tensor.matmul(
                        out, pp[p % 4][:, lhs[j] : lhs[j] + 128], rhss[j],
                        start=(j == 0), stop=(j == 5),
                    )
                    if blk == 0 and j == 0 and gate_act is not None:
                        mm.wait_op(act, gate_act, GE)  # pp ready
                    if blk == 1 and j == 5 and p >= NP - 2:
                        mm.then_inc(z, 1)  # tail markers for the last copies

        @block.tensor
        def _(tensor):
            tensor.wait_ge(diK[0], 16 * K_TIER[0])
            tensor.wait_ge(mz[0], 16 * 3)  # buffer-0 upper rows zeroed
            emit_scores(0)
            emit_scores(1)
            for p in range(NP):
                s, hb = divmod(p, PPS)
                if p + 2 < NP:
                    s2, hb2 = divmod(p + 2, PPS)
                    if s2 == 0 and hb2 == 2:
                        tensor.wait_ge(diK[1], 16 * K_TIER[1])
                    elif s2 == 0 and hb2 == PRE_P:
                        tensor.wait_ge(diK[2], 16 * K_TIER[2])
                    elif hb2 == 0 and s2 > 0:
                        tensor.wait_ge(diK[s2 + 2], 16 * K_TIER[s2 + 2])
                        if s2 == 1:
                            tensor.wait_ge(mz[1], 16 * 3)  # buffer-1 zeroed
                    emit_scores(p + 2, gate_act=p + 1)
                if s == 0 and hb == 0:
                    tensor.wait_ge(diV[0], 16 * V_TIER[0])
                elif s == 0 and hb == 2:
                    tensor.wait_ge(diV[1], 16 * V_TIER[1])
                elif s == 0 and hb == PRE_P:
                    tensor.wait_ge(diV[2], 16 * V_TIER[2])
                elif hb == 0:
                    tensor.wait_ge(diV[s + 2], 16 * V_TIER[s + 2])
                emit_pv(p, gate_act=p + 1 if p + 2 >= NP else None)

        @block.scalar
        def _(scalar):
            # zero the garbage upper rows of the [64]-loaded K tensors once
            # (their cols are multiplied by qt's zero rows, but leftover SBUF
            # bits could be NaN/inf patterns and NaN*0 = NaN).
            for u2 in range(2):
                for dst, w in ((lkt_t[u2], LKT_W), (skt_t[u2], SKT_W),
                               (gkt_t[u2], 128)):
                    scalar.dma_start(
                        dst[64:128, :], kz[:, 0:w]
                    ).then_inc(mz[u2], 16)
            for p in range(NP):
                u = p % 2
                nc.scalar.activation(
                    pp[p % 4][:, 512:1536], psA[u][:, 0:1024], EXP, scale=0.125
                ).wait_op(zs, p + 1, GE).then_inc(act, 1)

        def emit_schrau(p, gate_zs=None):
            ts = nc.vector.tensor_scalar(
                pp[p % 4][:, 0:512].bitcast(I16),
                psB[p % 2][:, 0:512],
                SCHRAU_MUL, SCHRAU_ADD,
                op0=mybir.AluOpType.mult, op1=mybir.AluOpType.add,
            )
            if gate_zs is not None:
                ts.wait_op(zs, gate_zs, GE)
            ts.then_inc(dact, 1)

        @block.vector
        def _(vector):
            emit_schrau(0, gate_zs=1)
            emit_schrau(1, gate_zs=2)
            for p in range(NP):
                u = p % 2
                u4 = p % 4
                if p >= 4:
                    vector.wait_ge(st, 16 * (p - 3))  # ob[u4] free
                pv3 = pv[u][:].rearrange("q (a c) -> q a c", a=4)
                cp = nc.vector.tensor_copy(ob[u4][:], pv3[:, 0:2, 0:65])
                if p <= NP - 4:
                    cp.wait_op(zs, p + 4, GE)   # scores(p+3) done => PV(p) done
                else:
                    cp.wait_op(z, 1 if p <= NP - 2 else 2, GE)
                cp.then_inc(dve, 1)
                if p + 2 < NP:
                    emit_schrau(p + 2)

        @block.gpsimd
        def _(gpsimd):
            for p in range(NP):
                s, hb = divmod(p, PPS)
                b = 2 * hb
                dst = o[s, b * B : (b + 2) * B, :].rearrange(
                    "(blk q) d -> q blk d", blk=2
                )
                gpsimd.dma_start(dst, ob[p % 4][:]).wait_op(
                    dve, p + 1, GE
                ).then_inc(st, 16)

    return nc


def _prepare(inputs):
    import ml_dtypes

    bf = ml_dtypes.bfloat16
    f = np.float32
    q = np.asarray(inputs["query_layer"], f).reshape(NH, T, D)
    k = np.asarray(inputs["key_layer"], f).reshape(NH, T, D)
    v = np.asarray(inputs["value_layer"], f).reshape(NH, T, D)
    sk = np.asarray(inputs["sparse_key"], f).reshape(NH, TSP, D)
    svv = np.asarray(inputs["sparse_value"], f).reshape(NH, TSP, D)
    gk = np.asarray(inputs["global_key"], f).reshape(NH, G, D)
    gvv = np.asarray(inputs["global_value"], f).reshape(NH, G, D)
    am = np.repeat(np.asarray(inputs["attention_mask"], f)[:, 0, 0, :], H, 0)
    sm = np.repeat(np.asarray(inputs["sparse_mask"], f)[:, 0, 0, :], H, 0)
    gm = np.repeat(np.asarray(inputs["global_mask"], f)[:, 0, 0, :], H, 0)

    qt = np.zeros((NH, 128, T), f)
    qt[:, :64] = q.transpose(0, 2, 1)
    qt = qt.astype(bf)

    lkt = np.zeros((NH, 64, LKT_W), f)
    lkt[:, :, B : B + T] = k.transpose(0, 2, 1)
    lkt = lkt.astype(bf)

    skt = np.zeros((NH, 64, SKT_W), f)
    skt[:, :, 160 : 160 + TSP] = sk.transpose(0, 2, 1)
    skt = skt.astype(bf)

    gkt = np.zeros((NH, 64, 128), f)
    gkt[:, :, :G] = gk.transpose(0, 2, 1)
    gkt = gkt.astype(bf)

    # V_aug rows scaled by exp(mask); pad rows are all-zero
    em_l = np.zeros((NH, LKT_W), f)
    em_l[:, B : B + T] = np.exp(am)
    lvp = np.zeros((NH, LKT_W, 65), f)
    lvp[:, B : B + T, :64] = v
    lvp[:, :, 64] = 1.0
    lvp *= em_l[:, :, None]
    lvp = np.ascontiguousarray(
        lvp.reshape(NH, LV_C, 128, 65).transpose(0, 2, 1, 3)
    ).reshape(NH, 128, LV_C * 65).astype(bf)

    SVP_W = 96 + SV_C * 128
    em_s = np.zeros((NH, SVP_W), f)
    em_s[:, 160 : 160 + TSP] = np.exp(sm)
    sv_pad = np.zeros((NH, SVP_W, 65), f)
    sv_pad[:, 160 : 160 + TSP, :64] = svv
    sv_pad[:, :, 64] = 1.0
    sv_pad *= em_s[:, :, None]
    svp = np.empty((NH, 4, 128, SV_C, 65), f)
    for p in range(4):
        svp[:, p] = (
            sv_pad[:, 32 * p : 32 * p + SV_C * 128]
            .reshape(NH, SV_C, 128, 65)
            .transpose(0, 2, 1, 3)
        )
    svp = np.ascontiguousarray(svp.transpose(0, 2, 1, 3, 4)).reshape(
        NH, 128, 4 * SV_C * 65
    ).astype(bf)

    gvp = np.zeros((NH, 128, 65), f)
    gvp[:, :G, :64] = gvv
    gvp[:, :G, 64] = 1.0
    gvp[:, :G] *= np.exp(gm)[:, :, None]
    gvp = gvp.astype(bf)
    kzz = np.zeros((64, LKT_W), bf)

    return [
        {
            "qt": qt[c * SL : (c + 1) * SL],
            "lkt": lkt[c * SL : (c + 1) * SL],
            "skt": skt[c * SL : (c + 1) * SL],
            "gkt": gkt[c * SL : (c + 1) * SL],
            "lv": lvp[c * SL : (c + 1) * SL],
            "sv": svp[c * SL : (c + 1) * SL],
            "gv": gvp[c * SL : (c + 1) * SL],
            "kz": kzz,
        }
        for c in range(NCORES)
    ]


_NC_CACHE = {}
LAST_RESULTS = None


def kernel(**inputs):
    global LAST_RESULTS
    if "nc" not in _NC_CACHE:
        _NC_CACHE["nc"] = _build_bass()
    nc = _NC_CACHE["nc"]
    in_maps = _prepare(inputs)
    res = run_bass_kernel_spmd(nc, in_maps, core_ids=list(range(NCORES)))
    LAST_RESULTS = res
    out = np.empty((NH, T, D), np.float32)
    for c in range(NCORES):
        raw = res.results[c]["o"].astype(np.float32)  # [SL, T, 65]
        out[c * SL : (c + 1) * SL] = raw[:, :, :64] / raw[:, :, 64:65]
    return out.reshape(N, H, T, D)


# revision 25
# speedup vs baseline: 1.2147x; 1.1490x over previous
"""Block-local sparse attention (LSG-style) on 8 TRN2 NeuronCores.

Sharding: the 32 (n, h) pairs are split 4-per-core (data/head parallel, no
collectives). Host-side numpy prep re-lays-out the inputs so the device
kernel needs no transposes, all bf16:

  - qt : Q^T per head [64, T]
  - lkt/skt/gkt: local/sparse/global K^T, token-padded with zeros
  - lv/sv/gv: V with a ones column appended (col 64), chunked [128, c, 65],
    and every row scaled by exp(mask): softmax(QK/8 + m) @ V is computed as
    sum_t exp(s_t) e^{m_t} [V_t, 1], then a divide by the accumulated last
    column. sv holds 4 phase-shifted copies so the 32-token-granular sparse
    windows always start at partition 0.

The device processes query-block PAIRS: 9 score matmuls per pair into a
3-bank PSUM region [128, 1536], one wide exp(S/8) on ACT, 12 PV matmuls
(N=65) accumulating [q, V|Z], and a reciprocal-normalize on DVE to bf16.

v2 scheduling (PE stream-bound at 0.836 ns/col; everything else hides
under it):
  - One semaphore increment per pair: the batch [scores(p+2), PV(p)] ends
    with a single then_inc(z) on the last PV matmul. Each mm-attached inc
    stalls the in-order PE queue ~115 ns waiting for stream completion, so
    v1's two incs/pair cost ~230 ns/pair.
  - Output stores ride the GpSimd queue (sync only issues loads), one
    bf16 store per pair (v1: two fp32 stores on sync, which head-of-line
    blocked behind input-load bursts, causing ~8 us stalls and a 10 us
    drain tail).
  - Loads are issued per-head with a pairs-0..5 prefix split so the PE
    starts after ~0.5 MB instead of 5 MB (v1 startup stall: ~21 us).
"""

from contextlib import ExitStack

import numpy as np

import concourse.bass as bass
import concourse.mybir as mybir
from concourse.bass_utils import run_bass_kernel_spmd

N, H, T, D = 2, 16, 4096, 64
B = 128          # query block
NB = T // B      # 32
G = 64           # global tokens
TSP = T // 4     # sparse tokens (1024)
NH = N * H       # 32
NCORES = 8
SL = NH // NCORES  # 4 heads per core
NP = SL * NB // 2  # 64 block-pairs per core
PPS = NB // 2      # 16 pairs per slot (slot == head)

LKT_W = T + 2 * B            # 4352 padded local tokens
SKT_W = TSP + 320            # 1344 padded sparse tokens
LV_C = LKT_W // 128          # 34 local V chunks
SV_C = 11                    # sparse V chunks per phase

F32 = mybir.dt.float32
BF16 = mybir.dt.bfloat16
I16 = mybir.dt.int16
# bf16 Schraudolph exp for the global chunk: bits16 = S*128/(8*ln2) + (16256-7)
SCHRAU_MUL = 128.0 / (8.0 * 0.6931471805599453)
SCHRAU_ADD = 16256.0 - 7.0
GE = "sem-ge"

# column layout of the per-pair score/prob tile [128, 1536] (3 PSUM banks;
# regions never cross a 512-col bank boundary)
# pp (probability tile) column layout, [128, 1536] bf16:
C_SP1A, C_SP1B = 0, 128
C_SP2A, C_SP2B = 256, 384
C_LOC1 = 512     # 256 wide: local chunk b+1, both blocks
C_LOC2 = 768     # 256 wide: local chunk b+2, both blocks
C_LOC0 = 1024    # 128: local chunk b, block A only
C_LOC3 = 1152    # 128: local chunk b+3, block B only
C_G = 1280       # 256 wide: global chunk, both blocks (true exp on ACT)
# Raw scores are split across two PSUM tensors so the engines that read
# them never share a PSUM bank-pair (concurrent ACT+DVE reads of nearby
# banks abort the NEFF): psA [128,1024] x2 (banks 0-3) holds locals+G,
# read by ACT's exp; psB [128,512] x2 (banks 4-5) holds the sparse slots,
# read by DVE's Schraudolph; pv x2 (banks 6-7) holds PV output + is read
# by DVE's copy. psA cols = pp cols - 512; psB cols = pp cols.

# load split: prefix covers pairs 0..PRE_P-1 (blocks 0..2*PRE_P-1) of head 0
PRE_P = 6
PRE_QT = 2 * PRE_P * B                 # qt cols
PRE_LKT = (2 * PRE_P + 3) * B          # lkt cols
PRE_SKT = 32 * (2 * PRE_P - 1) + 352   # skt cols
PRE_LV = (2 * PRE_P + 3) * 65          # lv cols
PRE_SVC = (32 * (2 * PRE_P - 1) + 224) // 128 + 1  # sv chunks per phase


def _build_bass():
    nc = bass.Bass("TRN2", num_devices=NCORES, debug=False)

    qt = nc.dram_tensor("qt", [SL, 128, T], BF16, kind="ExternalInput")
    lkt = nc.dram_tensor("lkt", [SL, 64, LKT_W], BF16, kind="ExternalInput")
    skt = nc.dram_tensor("skt", [SL, 64, SKT_W], BF16, kind="ExternalInput")
    gkt = nc.dram_tensor("gkt", [SL, 64, 128], BF16, kind="ExternalInput")
    lv = nc.dram_tensor("lv", [SL, 128, LV_C * 65], BF16, kind="ExternalInput")
    sv = nc.dram_tensor("sv", [SL, 128, 4 * SV_C * 65], BF16, kind="ExternalInput")
    gv = nc.dram_tensor("gv", [SL, 128, 65], BF16, kind="ExternalInput")
    kz = nc.dram_tensor("kz", [64, LKT_W], BF16, kind="ExternalInput")  # zeros
    o = nc.dram_tensor("o", [SL, T, 65], BF16, kind="ExternalOutput")

    EXP = mybir.ActivationFunctionType.Exp

    with ExitStack() as es:
        ec = es.enter_context
        # double-buffered inputs (head parity)
        qt_t = [ec(nc.sbuf_tensor(f"qt_t{i}", [128, T], BF16)) for i in range(2)]
        lkt_t = [ec(nc.sbuf_tensor(f"lkt_t{i}", [128, LKT_W], BF16)) for i in range(2)]
        skt_t = [ec(nc.sbuf_tensor(f"skt_t{i}", [128, SKT_W], BF16)) for i in range(2)]
        gkt_t = [ec(nc.sbuf_tensor(f"gkt_t{i}", [128, 128], BF16)) for i in range(2)]
        lv_t = [ec(nc.sbuf_tensor(f"lv_t{i}", [128, LV_C * 65], BF16)) for i in range(2)]
        sv_t = [ec(nc.sbuf_tensor(f"sv_t{i}", [128, 4 * SV_C * 65], BF16)) for i in range(2)]
        gv_t = [ec(nc.sbuf_tensor(f"gv_t{i}", [128, 65], BF16)) for i in range(2)]
        # double-buffered per-pair working set (pair parity)
        psA = [ec(nc.psum_tensor(f"psA{i}", [128, 1024], F32)) for i in range(2)]  # banks 0-3
        psB = [ec(nc.psum_tensor(f"psB{i}", [128, 512], F32)) for i in range(2)]   # banks 4-5
        pv = [ec(nc.psum_tensor(f"pv{i}", [128, 512], F32)) for i in range(2)]     # banks 6-7
        pp = [ec(nc.sbuf_tensor(f"pp{i}", [128, 1536], BF16)) for i in range(4)]
        ob = [ec(nc.sbuf_tensor(f"ob{i}", [128, 2, 65], BF16)) for i in range(4)]

        # one semaphore per load tier: DMA completions are out-of-order
        # across engines, so a shared counter can hit its threshold with a
        # later DMA substituting for an unfinished earlier one.
        diK = [ec(nc.semaphore(f"diK{i}")) for i in range(6)]
        diV = [ec(nc.semaphore(f"diV{i}")) for i in range(6)]
        z = ec(nc.semaphore("z"))      # tail marker: +1 per PV of the last 2 batches
        zs = ec(nc.semaphore("zs"))    # +1 per completed scores(j): zs = j+1
        mz = [ec(nc.semaphore(f"mz{i}")) for i in range(2)]
        act = ec(nc.semaphore("act"))  # +1 per pair: ACT exp done
        dact = ec(nc.semaphore("dact"))  # +1 per pair: DVE schrau exp done
        dve = ec(nc.semaphore("dve"))  # +1 per pair: normalize done
        st = ec(nc.semaphore("st"))    # +16 per pair: store done
        block = ec(nc.Block(no_gpsimd_drain=True))

        # DMAs per tier (x16 per DMA):
        # K: Kpre0(4: pairs 0-1) Kpre1(3: ..pair 5) Krest(3) h1(4) h2(4) h3(4)
        # V: Vpre0(3: pairs 0-1) Vpre1(1: ..pair 5) Vrest(2) h1(3) h2(3) h3(3)
        K_TIER = [4, 3, 3, 4, 4, 4]
        V_TIER = [3, 1, 2, 3, 3, 3]

        @block.sync
        def _(sync):
            KP0_QT, KP1_QT = 512, 1536
            KP0_LKT, KP1_LKT = 896, PRE_LKT
            KP0_SKT, KP1_SKT = 448, PRE_SKT

            def load_head_k(s, u, sem):
                for dst, srcp in (
                    (qt_t[u][:], qt[s]),
                    (lkt_t[u][0:64, :], lkt[s]),
                    (skt_t[u][0:64, :], skt[s]),
                    (gkt_t[u][0:64, :], gkt[s]),
                ):
                    dma = sync.dma_start(dst, srcp)
                    dma.then_inc(sem, 16)
                    yield dma

            def load_head_v(s, u, sem):
                for dst, srcp in (
                    (lv_t[u][:], lv[s]),
                    (sv_t[u][:], sv[s]),
                    (gv_t[u][:], gv[s]),
                ):
                    dma = sync.dma_start(dst, srcp)
                    dma.then_inc(sem, 16)
                    yield dma

            # head 0, finely tiered
            for dst, srcp in (
                (qt_t[0][:, 0:KP0_QT], qt[0, :, 0:KP0_QT]),
                (lkt_t[0][0:64, 0:KP0_LKT], lkt[0, :, 0:KP0_LKT]),
                (skt_t[0][0:64, 0:KP0_SKT], skt[0, :, 0:KP0_SKT]),
                (gkt_t[0][0:64, :], gkt[0]),
            ):
                sync.dma_start(dst, srcp).then_inc(diK[0], 16)
            for dst, srcp in (
                (qt_t[0][:, KP0_QT:KP1_QT], qt[0, :, KP0_QT:KP1_QT]),
                (lkt_t[0][0:64, KP0_LKT:KP1_LKT], lkt[0, :, KP0_LKT:KP1_LKT]),
                (skt_t[0][0:64, KP0_SKT:KP1_SKT], skt[0, :, KP0_SKT:KP1_SKT]),
            ):
                sync.dma_start(dst, srcp).then_inc(diK[1], 16)
            sv3 = sv_t[0][:].rearrange("p (ph c) -> p ph c", ph=4)
            sv3s = sv[0].rearrange("p (ph c) -> p ph c", ph=4)
            for dst, srcp in (
                (lv_t[0][:, 0:PRE_LV], lv[0, :, 0:PRE_LV]),
                (sv3[:, :, 0 : 3 * 65], sv3s[:, :, 0 : 3 * 65]),
                (gv_t[0][:], gv[0]),
            ):
                sync.dma_start(dst, srcp).then_inc(diV[0], 16)
            sync.dma_start(
                sv3[:, :, 3 * 65 : 5 * 65], sv3s[:, :, 3 * 65 : 5 * 65]
            ).then_inc(diV[1], 16)
            for dst, srcp in (
                (qt_t[0][:, KP1_QT:T], qt[0, :, KP1_QT:T]),
                (lkt_t[0][0:64, KP1_LKT:LKT_W], lkt[0, :, KP1_LKT:LKT_W]),
                (skt_t[0][0:64, KP1_SKT:SKT_W], skt[0, :, KP1_SKT:SKT_W]),
            ):
                sync.dma_start(dst, srcp).then_inc(diK[2], 16)
            for dst, srcp in (
                (lv_t[0][:, PRE_LV:], lv[0, :, PRE_LV:]),
                (sv3[:, :, 5 * 65 :], sv3s[:, :, 5 * 65 :]),
            ):
                sync.dma_start(dst, srcp).then_inc(diV[2], 16)
            # head 1 unconditionally
            list(load_head_k(1, 1, diK[3]))
            list(load_head_v(1, 1, diV[3]))
            # heads 2 and 3 gated on scores/PV completion (zs markers)
            for first, dma in enumerate(load_head_k(2, 0, diK[4])):
                if first == 0:
                    dma.wait_op(zs, PPS, GE)
            for first, dma in enumerate(load_head_v(2, 0, diV[4])):
                if first == 0:
                    dma.wait_op(zs, PPS + 3, GE)
            for first, dma in enumerate(load_head_k(3, 1, diK[5])):
                if first == 0:
                    dma.wait_op(zs, 2 * PPS, GE)
            for first, dma in enumerate(load_head_v(3, 1, diV[5])):
                if first == 0:
                    dma.wait_op(zs, 2 * PPS + 3, GE)
            sync.wait_ge(st, 16 * NP)

        def emit_scores(p, gate_act=None):
            s, hb = divmod(p, PPS)
            b = 2 * hb
            u = p % 2
            su = s % 2
            qA = qt_t[su][:, b * B : (b + 1) * B]
            qB = qt_t[su][:, (b + 1) * B : (b + 2) * B]
            qAB = qt_t[su][:, b * B : (b + 2) * B]
            w1a, w2a = 32 * b, 32 * b + 224
            w1b, w2b = w1a + 32, w2a + 32
            # ACT-region (psA) matmuls first: the act-gate rides mm #1;
            # the sparse (psB, DVE-read) matmuls come after the tucked
            # dact-wait so they never overwrite what the DVE still reads.
            mms = (
                (C_LOC1, 256, lkt_t[su][:, (b + 1) * B : (b + 2) * B], qAB),
                (C_LOC0, 128, lkt_t[su][:, b * B : (b + 1) * B], qA),
                (C_LOC2, 256, lkt_t[su][:, (b + 2) * B : (b + 3) * B], qAB),
                (C_LOC3, 128, lkt_t[su][:, (b + 3) * B : (b + 4) * B], qB),
                (C_G, 256, gkt_t[su][:, :], qAB),
                (C_SP1A, 128, skt_t[su][:, w1a : w1a + 128], qA),
                (C_SP1B, 128, skt_t[su][:, w1b : w1b + 128], qB),
                (C_SP2A, 128, skt_t[su][:, w2a : w2a + 128], qA),
                (C_SP2B, 128, skt_t[su][:, w2b : w2b + 128], qB),
            )
            for kk, (col, w, lhsT, rhs) in enumerate(mms):
                out_ap = (psB[u][:, col : col + w] if col < 512
                          else psA[u][:, col - 512 : col - 512 + w])
                mm = nc.tensor.matmul(
                    out_ap,
                    lhsT, rhs,
                    start=True, stop=True,
                )
                if kk == 0 and gate_act is not None:
                    mm.wait_op(act, gate_act, GE)  # psS[u] free (exp done)
                if kk == 0 and gate_act is not None:
                    # pv free + schrau(p-2) done, checked here (pre-satisfied)
                    # so the queue checks hide under this matmul's stream.
                    if p >= 4:
                        nc.tensor.wait_ge(dve, p - 3)
                    if p >= 2:
                        nc.tensor.wait_ge(dact, p - 1)
                if kk == len(mms) - 1:
                    mm.then_inc(zs, 1)

        def emit_pv(p, gate_act=None):
            s, hb = divmod(p, PPS)
            b = 2 * hb
            u = p % 2
            su = s % 2
            if gate_act is not None and p >= 2:
                # no scores batch ahead of this PV to carry the act gate;
                # the dve wait moves to a standalone so the first mm can
                # carry the act wait (one sem wait per instruction).
                nc.tensor.wait_ge(dve, p - 1)
                nc.tensor.wait_ge(dact, p + 1)
            for blk in range(2):
                bb = b + blk
                w1, w2 = 32 * bb, 32 * bb + 224
                c1, r1 = divmod(w1, 128)
                c2, r2 = divmod(w2, 128)
                sp1c = ((r1 // 32) * SV_C + c1) * 65
                sp2c = ((r2 // 32) * SV_C + c2) * 65
                if blk == 0:
                    lhs = (C_SP1A, C_SP2A, C_G, C_LOC0, C_LOC1, C_LOC2)
                else:
                    lhs = (C_SP1B, C_SP2B, C_G + 128, C_LOC1 + 128,
                           C_LOC2 + 128, C_LOC3)
                # (layout-independent: cols come from the C_* constants)
                rhss = (
                    sv_t[su][:, sp1c : sp1c + 65],
                    sv_t[su][:, sp2c : sp2c + 65],
                    gv_t[su][:],
                    lv_t[su][:, bb * 65 : bb * 65 + 65],
                    lv_t[su][:, (bb + 1) * 65 : (bb + 1) * 65 + 65],
                    lv_t[su][:, (bb + 2) * 65 : (bb + 2) * 65 + 65],
                )
                out = pv[u][:, blk * 128 : blk * 128 + 65]
                for j in range(6):
                    mm = nc.tensor.matmul(
                        out, pp[p % 4][:, lhs[j] : lhs[j] + 128], rhss[j],
                        start=(j == 0), stop=(j == 5),
                    )
                    if blk == 0 and j == 0 and gate_act is not None:
                        mm.wait_op(act, gate_act, GE)  # pp ready
                    if blk == 1 and j == 5 and p >= NP - 2:
                        mm.then_inc(z, 1)  # tail markers for the last copies

        @block.tensor
        def _(tensor):
            tensor.wait_ge(diK[0], 16 * K_TIER[0])
            tensor.wait_ge(mz[0], 16 * 3)  # buffer-0 upper rows zeroed
            emit_scores(0)
            emit_scores(1)
            for p in range(NP):
                s, hb = divmod(p, PPS)
                if p + 2 < NP:
                    s2, hb2 = divmod(p + 2, PPS)
                    if s2 == 0 and hb2 == 2:
                        tensor.wait_ge(diK[1], 16 * K_TIER[1])
                    elif s2 == 0 and hb2 == PRE_P:
                        tensor.wait_ge(diK[2], 16 * K_TIER[2])
                    elif hb2 == 0 and s2 > 0:
                        tensor.wait_ge(diK[s2 + 2], 16 * K_TIER[s2 + 2])
                        if s2 == 1:
                            tensor.wait_ge(mz[1], 16 * 3)  # buffer-1 zeroed
                    emit_scores(p + 2, gate_act=p + 1)
                if s == 0 and hb == 0:
                    tensor.wait_ge(diV[0], 16 * V_TIER[0])
                elif s == 0 and hb == 2:
                    tensor.wait_ge(diV[1], 16 * V_TIER[1])
                elif s == 0 and hb == PRE_P:
                    tensor.wait_ge(diV[2], 16 * V_TIER[2])
                elif hb == 0:
                    tensor.wait_ge(diV[s + 2], 16 * V_TIER[s + 2])
                emit_pv(p, gate_act=p + 1 if p + 2 >= NP else None)

        @block.scalar
        def _(scalar):
            # zero the garbage upper rows of the [64]-loaded K tensors once
            # (their cols are multiplied by qt's zero rows, but leftover SBUF
            # bits could be NaN/inf patterns and NaN*0 = NaN).
            for u2 in range(2):
                for dst, w in ((lkt_t[u2], LKT_W), (skt_t[u2], SKT_W),
                               (gkt_t[u2], 128)):
                    scalar.dma_start(
                        dst[64:128, :], kz[:, 0:w]
                    ).then_inc(mz[u2], 16)
            for p in range(NP):
                u = p % 2
                nc.scalar.activation(
                    pp[p % 4][:, 512:1536], psA[u][:, 0:1024], EXP, scale=0.125
                ).wait_op(zs, p + 1, GE).then_inc(act, 1)

        def emit_schrau(p, gate_zs=None):
            ts = nc.vector.tensor_scalar(
                pp[p % 4][:, 0:512].bitcast(I16),
                psB[p % 2][:, 0:512],
                SCHRAU_MUL, SCHRAU_ADD,
                op0=mybir.AluOpType.mult, op1=mybir.AluOpType.add,
            )
            if gate_zs is not None:
                ts.wait_op(zs, gate_zs, GE)
            ts.then_inc(dact, 1)

        @block.vector
        def _(vector):
            emit_schrau(0, gate_zs=1)
            emit_schrau(1, gate_zs=2)
            for p in range(NP):
                u = p % 2
                u4 = p % 4
                if p + 2 < NP:
                    # schrau(p+2) first, on its own earlier gate, so dact
                    # lands a full period before batch p+4 checks it.
                    emit_schrau(p + 2, gate_zs=p + 3)
                if p >= 4:
                    vector.wait_ge(st, 16 * (p - 3))  # ob[u4] free
                pv3 = pv[u][:].rearrange("q (a c) -> q a c", a=4)
                cp = nc.vector.tensor_copy(ob[u4][:], pv3[:, 0:2, 0:65])
                if p <= NP - 4:
                    cp.wait_op(zs, p + 4, GE)   # scores(p+3) done => PV(p) done
                else:
                    cp.wait_op(z, 1 if p <= NP - 2 else 2, GE)
                cp.then_inc(dve, 1)

        @block.gpsimd
        def _(gpsimd):
            for p in range(NP):
                s, hb = divmod(p, PPS)
                b = 2 * hb
                dst = o[s, b * B : (b + 2) * B, :].rearrange(
                    "(blk q) d -> q blk d", blk=2
                )
                gpsimd.dma_start(dst, ob[p % 4][:]).wait_op(
                    dve, p + 1, GE
                ).then_inc(st, 16)

    return nc


def _prepare(inputs):
    import ml_dtypes

    bf = ml_dtypes.bfloat16
    f = np.float32
    q = np.asarray(inputs["query_layer"], f).reshape(NH, T, D)
    k = np.asarray(inputs["key_layer"], f).reshape(NH, T, D)
    v = np.asarray(inputs["value_layer"], f).reshape(NH, T, D)
    sk = np.asarray(inputs["sparse_key"], f).reshape(NH, TSP, D)
    svv = np.asarray(inputs["sparse_value"], f).reshape(NH, TSP, D)
    gk = np.asarray(inputs["global_key"], f).reshape(NH, G, D)
    gvv = np.asarray(inputs["global_value"], f).reshape(NH, G, D)
    am = np.repeat(np.asarray(inputs["attention_mask"], f)[:, 0, 0, :], H, 0)
    sm = np.repeat(np.asarray(inputs["sparse_mask"], f)[:, 0, 0, :], H, 0)
    gm = np.repeat(np.asarray(inputs["global_mask"], f)[:, 0, 0, :], H, 0)

    qt = np.zeros((NH, 128, T), f)
    qt[:, :64] = q.transpose(0, 2, 1)
    qt = qt.astype(bf)

    lkt = np.zeros((NH, 64, LKT_W), f)
    lkt[:, :, B : B + T] = k.transpose(0, 2, 1)
    lkt = lkt.astype(bf)

    skt = np.zeros((NH, 64, SKT_W), f)
    skt[:, :, 160 : 160 + TSP] = sk.transpose(0, 2, 1)
    skt = skt.astype(bf)

    gkt = np.zeros((NH, 64, 128), f)
    gkt[:, :, :G] = gk.transpose(0, 2, 1)
    gkt = gkt.astype(bf)

    # V_aug rows scaled by exp(mask); pad rows are all-zero
    em_l = np.zeros((NH, LKT_W), f)
    em_l[:, B : B + T] = np.exp(am)
    lvp = np.zeros((NH, LKT_W, 65), f)
    lvp[:, B : B + T, :64] = v
    lvp[:, :, 64] = 1.0
    lvp *= em_l[:, :, None]
    lvp = np.ascontiguousarray(
        lvp.reshape(NH, LV_C, 128, 65).transpose(0, 2, 1, 3)
    ).reshape(NH, 128, LV_C * 65).astype(bf)

    SVP_W = 96 + SV_C * 128
    em_s = np.zeros((NH, SVP_W), f)
    em_s[:, 160 : 160 + TSP] = np.exp(sm)
    sv_pad = np.zeros((NH, SVP_W, 65), f)
    sv_pad[:, 160 : 160 + TSP, :64] = svv
    sv_pad[:, :, 64] = 1.0
    sv_pad *= em_s[:, :, None]
    svp = np.empty((NH, 4, 128, SV_C, 65), f)
    for p in range(4):
        svp[:, p] = (
            sv_pad[:, 32 * p : 32 * p + SV_C * 128]
            .reshape(NH, SV_C, 128, 65)
            .transpose(0, 2, 1, 3)
        )
    svp = np.ascontiguousarray(svp.transpose(0, 2, 1, 3, 4)).reshape(
        NH, 128, 4 * SV_C * 65
    ).astype(bf)

    gvp = np.zeros((NH, 128, 65), f)
    gvp[:, :G, :64] = gvv
    gvp[:, :G, 64] = 1.0
    gvp[:, :G] *= np.exp(gm)[:, :, None]
    gvp = gvp.astype(bf)
    kzz = np.zeros((64, LKT_W), bf)

    return [
        {
            "qt": qt[c * SL : (c + 1) * SL],
            "lkt": lkt[c * SL : (c + 1) * SL],
            "skt": skt[c * SL : (c + 1) * SL],
            "gkt": gkt[c * SL : (c + 1) * SL],
            "lv": lvp[c * SL : (c + 1) * SL],
            "sv": svp[c * SL : (c + 1) * SL],
            "gv": gvp[c * SL : (c + 1) * SL],
            "kz": kzz,
        }
        for c in range(NCORES)
    ]


_NC_CACHE = {}
LAST_RESULTS = None


def kernel(**inputs):
    global LAST_RESULTS
    if "nc" not in _NC_CACHE:
        _NC_CACHE["nc"] = _build_bass()
    nc = _NC_CACHE["nc"]
    in_maps = _prepare(inputs)
    res = run_bass_kernel_spmd(nc, in_maps, core_ids=list(range(NCORES)))
    LAST_RESULTS = res
    out = np.empty((NH, T, D), np.float32)
    for c in range(NCORES):
        raw = res.results[c]["o"].astype(np.float32)  # [SL, T, 65]
        out[c * SL : (c + 1) * SL] = raw[:, :, :64] / raw[:, :, 64:65]
    return out.reshape(N, H, T, D)


# revision 26
# speedup vs baseline: 1.2823x; 1.0556x over previous
"""Block-local sparse attention (LSG-style) on 8 TRN2 NeuronCores.

Sharding: the 32 (n, h) pairs are split 4-per-core (data/head parallel, no
collectives). Host-side numpy prep re-lays-out the inputs so the device
kernel needs no transposes, all bf16:

  - qt : Q^T per head [64, T]
  - lkt/skt/gkt: local/sparse/global K^T, token-padded with zeros
  - lv/sv/gv: V with a ones column appended (col 64), chunked [128, c, 65],
    and every row scaled by exp(mask): softmax(QK/8 + m) @ V is computed as
    sum_t exp(s_t) e^{m_t} [V_t, 1], then a divide by the accumulated last
    column. sv holds 4 phase-shifted copies so the 32-token-granular sparse
    windows always start at partition 0.

The device processes query-block PAIRS: 9 score matmuls per pair into a
3-bank PSUM region [128, 1536], one wide exp(S/8) on ACT, 12 PV matmuls
(N=65) accumulating [q, V|Z], and a reciprocal-normalize on DVE to bf16.

v2 scheduling (PE stream-bound at 0.836 ns/col; everything else hides
under it):
  - One semaphore increment per pair: the batch [scores(p+2), PV(p)] ends
    with a single then_inc(z) on the last PV matmul. Each mm-attached inc
    stalls the in-order PE queue ~115 ns waiting for stream completion, so
    v1's two incs/pair cost ~230 ns/pair.
  - Output stores ride the GpSimd queue (sync only issues loads), one
    bf16 store per pair (v1: two fp32 stores on sync, which head-of-line
    blocked behind input-load bursts, causing ~8 us stalls and a 10 us
    drain tail).
  - Loads are issued per-head with a pairs-0..5 prefix split so the PE
    starts after ~0.5 MB instead of 5 MB (v1 startup stall: ~21 us).
"""

from contextlib import ExitStack

import numpy as np

import concourse.bass as bass
import concourse.mybir as mybir
from concourse.bass_utils import run_bass_kernel_spmd

N, H, T, D = 2, 16, 4096, 64
B = 128          # query block
NB = T // B      # 32
G = 64           # global tokens
TSP = T // 4     # sparse tokens (1024)
NH = N * H       # 32
NCORES = 8
SL = NH // NCORES  # 4 heads per core
NP = SL * NB // 2  # 64 block-pairs per core
PPS = NB // 2      # 16 pairs per slot (slot == head)

LKT_W = T + 2 * B            # 4352 padded local tokens
SKT_W = TSP + 320            # 1344 padded sparse tokens
LV_C = LKT_W // 128          # 34 local V chunks
SV_C = 11                    # sparse V chunks per phase

F32 = mybir.dt.float32
BF16 = mybir.dt.bfloat16
I16 = mybir.dt.int16
# bf16 Schraudolph exp for the global chunk: bits16 = S*128/(8*ln2) + (16256-7)
SCHRAU_MUL = 128.0 / (8.0 * 0.6931471805599453)
SCHRAU_ADD = 16256.0 - 7.0
GE = "sem-ge"

# column layout of the per-pair score/prob tile [128, 1536] (3 PSUM banks;
# regions never cross a 512-col bank boundary)
# pp (probability tile) column layout, [128, 1536] bf16:
C_SP1A, C_SP1B = 0, 128
C_SP2A, C_SP2B = 256, 384
C_LOC1 = 512     # 256 wide: local chunk b+1, both blocks
C_LOC2 = 768     # 256 wide: local chunk b+2, both blocks
C_LOC0 = 1024    # 128: local chunk b, block A only
C_LOC3 = 1152    # 128: local chunk b+3, block B only
C_G = 1280       # 256 wide: global chunk, both blocks (true exp on ACT)
# Raw scores are split across two PSUM tensors so the engines that read
# them never share a PSUM bank-pair (concurrent ACT+DVE reads of nearby
# banks abort the NEFF): psA [128,1024] x2 (banks 0-3) holds locals+G,
# read by ACT's exp; psB [128,512] x2 (banks 4-5) holds the sparse slots,
# read by DVE's Schraudolph; pv x2 (banks 6-7) holds PV output + is read
# by DVE's copy. psA cols = pp cols - 512; psB cols = pp cols.

# load split: prefix covers pairs 0..PRE_P-1 (blocks 0..2*PRE_P-1) of head 0
PRE_P = 6
PRE_QT = 2 * PRE_P * B                 # qt cols
PRE_LKT = (2 * PRE_P + 3) * B          # lkt cols
PRE_SKT = 32 * (2 * PRE_P - 1) + 352   # skt cols
PRE_LV = (2 * PRE_P + 3) * 65          # lv cols
PRE_SVC = (32 * (2 * PRE_P - 1) + 224) // 128 + 1  # sv chunks per phase


def _build_bass():
    nc = bass.Bass("TRN2", num_devices=NCORES, debug=False)

    qt = nc.dram_tensor("qt", [SL, 128, T], BF16, kind="ExternalInput")
    lkt = nc.dram_tensor("lkt", [SL, 64, LKT_W], BF16, kind="ExternalInput")
    skt = nc.dram_tensor("skt", [SL, 64, SKT_W], BF16, kind="ExternalInput")
    gkt = nc.dram_tensor("gkt", [SL, 64, 128], BF16, kind="ExternalInput")
    lv = nc.dram_tensor("lv", [SL, 128, LV_C * 65], BF16, kind="ExternalInput")
    sv = nc.dram_tensor("sv", [SL, 128, 4 * SV_C * 65], BF16, kind="ExternalInput")
    gv = nc.dram_tensor("gv", [SL, 128, 65], BF16, kind="ExternalInput")
    kz = nc.dram_tensor("kz", [64, LKT_W], BF16, kind="ExternalInput")  # zeros
    o = nc.dram_tensor("o", [SL, T, 65], BF16, kind="ExternalOutput")

    EXP = mybir.ActivationFunctionType.Exp

    with ExitStack() as es:
        ec = es.enter_context
        # double-buffered inputs (head parity)
        qt_t = [ec(nc.sbuf_tensor(f"qt_t{i}", [128, T], BF16)) for i in range(2)]
        lkt_t = [ec(nc.sbuf_tensor(f"lkt_t{i}", [128, LKT_W], BF16)) for i in range(2)]
        skt_t = [ec(nc.sbuf_tensor(f"skt_t{i}", [128, SKT_W], BF16)) for i in range(2)]
        gkt_t = [ec(nc.sbuf_tensor(f"gkt_t{i}", [128, 128], BF16)) for i in range(2)]
        lv_t = [ec(nc.sbuf_tensor(f"lv_t{i}", [128, LV_C * 65], BF16)) for i in range(2)]
        sv_t = [ec(nc.sbuf_tensor(f"sv_t{i}", [128, 4 * SV_C * 65], BF16)) for i in range(2)]
        gv_t = [ec(nc.sbuf_tensor(f"gv_t{i}", [128, 65], BF16)) for i in range(2)]
        # double-buffered per-pair working set (pair parity)
        psA = [ec(nc.psum_tensor(f"psA{i}", [128, 1024], F32)) for i in range(2)]  # banks 0-3
        psB = [ec(nc.psum_tensor(f"psB{i}", [128, 512], F32)) for i in range(2)]   # banks 4-5
        pv = [ec(nc.psum_tensor(f"pv{i}", [128, 512], F32)) for i in range(2)]     # banks 6-7
        pp = [ec(nc.sbuf_tensor(f"pp{i}", [128, 1536], BF16)) for i in range(4)]
        ob = [ec(nc.sbuf_tensor(f"ob{i}", [128, 2, 65], BF16)) for i in range(8)]

        # one semaphore per load tier: DMA completions are out-of-order
        # across engines, so a shared counter can hit its threshold with a
        # later DMA substituting for an unfinished earlier one.
        diK = [ec(nc.semaphore(f"diK{i}")) for i in range(6)]
        diV = [ec(nc.semaphore(f"diV{i}")) for i in range(6)]
        z = ec(nc.semaphore("z"))      # tail marker: +1 per PV of the last 2 batches
        zs = ec(nc.semaphore("zs"))    # +1 per completed scores(j): zs = j+1
        mz = [ec(nc.semaphore(f"mz{i}")) for i in range(3)]
        act = ec(nc.semaphore("act"))  # +1 per pair: ACT exp done
        dact = ec(nc.semaphore("dact"))  # +1 per pair: DVE schrau exp done
        dve = ec(nc.semaphore("dve"))  # +1 per pair: normalize done
        st = ec(nc.semaphore("st"))    # +16 per pair: store done
        block = ec(nc.Block(no_gpsimd_drain=True))

        # DMAs per tier (x16 per DMA):
        # K: Kpre0(4: pairs 0-1) Kpre1(3: ..pair 5) Krest(3) h1(4) h2(4) h3(4)
        # V: Vpre0(3: pairs 0-1) Vpre1(1: ..pair 5) Vrest(2) h1(3) h2(3) h3(3)
        K_TIER = [4, 3, 3, 4, 4, 4]
        V_TIER = [3, 1, 2, 3, 3, 3]

        @block.sync
        def _(sync):
            KP0_QT, KP1_QT = 512, 1536
            KP0_LKT, KP1_LKT = 896, PRE_LKT
            KP0_SKT, KP1_SKT = 448, PRE_SKT

            def load_head_k(s, u, sem):
                for dst, srcp in (
                    (qt_t[u][:], qt[s]),
                    (lkt_t[u][0:64, :], lkt[s]),
                    (skt_t[u][0:64, :], skt[s]),
                    (gkt_t[u][0:64, :], gkt[s]),
                ):
                    dma = sync.dma_start(dst, srcp)
                    dma.then_inc(sem, 16)
                    yield dma

            def load_head_v(s, u, sem):
                for dst, srcp in (
                    (lv_t[u][:], lv[s]),
                    (sv_t[u][:], sv[s]),
                    (gv_t[u][:], gv[s]),
                ):
                    dma = sync.dma_start(dst, srcp)
                    dma.then_inc(sem, 16)
                    yield dma

            # head 0, finely tiered
            for dst, srcp in (
                (qt_t[0][:, 0:KP0_QT], qt[0, :, 0:KP0_QT]),
                (lkt_t[0][0:64, 0:KP0_LKT], lkt[0, :, 0:KP0_LKT]),
                (skt_t[0][0:64, 0:KP0_SKT], skt[0, :, 0:KP0_SKT]),
                (gkt_t[0][0:64, :], gkt[0]),
            ):
                sync.dma_start(dst, srcp).then_inc(diK[0], 16)
            for dst, srcp in (
                (qt_t[0][:, KP0_QT:KP1_QT], qt[0, :, KP0_QT:KP1_QT]),
                (lkt_t[0][0:64, KP0_LKT:KP1_LKT], lkt[0, :, KP0_LKT:KP1_LKT]),
                (skt_t[0][0:64, KP0_SKT:KP1_SKT], skt[0, :, KP0_SKT:KP1_SKT]),
            ):
                sync.dma_start(dst, srcp).then_inc(diK[1], 16)
            sv3 = sv_t[0][:].rearrange("p (ph c) -> p ph c", ph=4)
            sv3s = sv[0].rearrange("p (ph c) -> p ph c", ph=4)
            for dst, srcp in (
                (lv_t[0][:, 0:PRE_LV], lv[0, :, 0:PRE_LV]),
                (sv3[:, :, 0 : 3 * 65], sv3s[:, :, 0 : 3 * 65]),
                (gv_t[0][:], gv[0]),
            ):
                sync.dma_start(dst, srcp).then_inc(diV[0], 16)
            sync.dma_start(
                sv3[:, :, 3 * 65 : 5 * 65], sv3s[:, :, 3 * 65 : 5 * 65]
            ).then_inc(diV[1], 16)
            for dst, srcp in (
                (qt_t[0][:, KP1_QT:T], qt[0, :, KP1_QT:T]),
                (lkt_t[0][0:64, KP1_LKT:LKT_W], lkt[0, :, KP1_LKT:LKT_W]),
                (skt_t[0][0:64, KP1_SKT:SKT_W], skt[0, :, KP1_SKT:SKT_W]),
            ):
                sync.dma_start(dst, srcp).then_inc(diK[2], 16)
            for dst, srcp in (
                (lv_t[0][:, PRE_LV:], lv[0, :, PRE_LV:]),
                (sv3[:, :, 5 * 65 :], sv3s[:, :, 5 * 65 :]),
            ):
                sync.dma_start(dst, srcp).then_inc(diV[2], 16)
            # head 1 unconditionally
            list(load_head_k(1, 1, diK[3]))
            list(load_head_v(1, 1, diV[3]))
            # heads 2 and 3 gated on scores/PV completion (zs markers)
            for first, dma in enumerate(load_head_k(2, 0, diK[4])):
                if first == 0:
                    dma.wait_op(zs, PPS, GE)
            for first, dma in enumerate(load_head_v(2, 0, diV[4])):
                if first == 0:
                    dma.wait_op(zs, PPS + 3, GE)
            for first, dma in enumerate(load_head_k(3, 1, diK[5])):
                if first == 0:
                    dma.wait_op(zs, 2 * PPS, GE)
            for first, dma in enumerate(load_head_v(3, 1, diV[5])):
                if first == 0:
                    dma.wait_op(zs, 2 * PPS + 3, GE)
            sync.wait_ge(st, 16 * NP)

        def emit_scores(p, gate_act=None):
            s, hb = divmod(p, PPS)
            b = 2 * hb
            u = p % 2
            su = s % 2
            qA = qt_t[su][:, b * B : (b + 1) * B]
            qB = qt_t[su][:, (b + 1) * B : (b + 2) * B]
            qAB = qt_t[su][:, b * B : (b + 2) * B]
            w1a, w2a = 32 * b, 32 * b + 224
            w1b, w2b = w1a + 32, w2a + 32
            # ACT-region (psA) matmuls first: the act-gate rides mm #1;
            # the sparse (psB, DVE-read) matmuls come after the tucked
            # dact-wait so they never overwrite what the DVE still reads.
            mms = (
                (C_LOC1, 256, lkt_t[su][:, (b + 1) * B : (b + 2) * B], qAB),
                (C_LOC0, 128, lkt_t[su][:, b * B : (b + 1) * B], qA),
                (C_LOC2, 256, lkt_t[su][:, (b + 2) * B : (b + 3) * B], qAB),
                (C_LOC3, 128, lkt_t[su][:, (b + 3) * B : (b + 4) * B], qB),
                (C_G, 256, gkt_t[su][:, :], qAB),
                (C_SP1A, 128, skt_t[su][:, w1a : w1a + 128], qA),
                (C_SP1B, 128, skt_t[su][:, w1b : w1b + 128], qB),
                (C_SP2A, 128, skt_t[su][:, w2a : w2a + 128], qA),
                (C_SP2B, 128, skt_t[su][:, w2b : w2b + 128], qB),
            )
            for kk, (col, w, lhsT, rhs) in enumerate(mms):
                out_ap = (psB[u][:, col : col + w] if col < 512
                          else psA[u][:, col - 512 : col - 512 + w])
                mm = nc.tensor.matmul(
                    out_ap,
                    lhsT, rhs,
                    start=True, stop=True,
                )
                if kk == 0 and gate_act is not None:
                    mm.wait_op(act, gate_act, GE)  # psS[u] free (exp done)
                if kk == 0 and gate_act is not None:
                    # pv free + schrau(p-2) done, checked here (pre-satisfied)
                    # so the queue checks hide under this matmul's stream.
                    if p >= 4:
                        nc.tensor.wait_ge(dve, p - 3)
                    if p >= 2:
                        nc.tensor.wait_ge(dact, p - 1)
                if kk == len(mms) - 1:
                    mm.then_inc(zs, 1)

        def emit_pv(p, gate_act=None):
            s, hb = divmod(p, PPS)
            b = 2 * hb
            u = p % 2
            su = s % 2
            if gate_act is not None and p >= 2:
                # no scores batch ahead of this PV to carry the act gate;
                # the dve wait moves to a standalone so the first mm can
                # carry the act wait (one sem wait per instruction).
                nc.tensor.wait_ge(dve, p - 1)
                nc.tensor.wait_ge(dact, p + 1)
            for blk in range(2):
                bb = b + blk
                w1, w2 = 32 * bb, 32 * bb + 224
                c1, r1 = divmod(w1, 128)
                c2, r2 = divmod(w2, 128)
                sp1c = ((r1 // 32) * SV_C + c1) * 65
                sp2c = ((r2 // 32) * SV_C + c2) * 65
                if blk == 0:
                    lhs = (C_SP1A, C_SP2A, C_G, C_LOC0, C_LOC1, C_LOC2)
                else:
                    lhs = (C_SP1B, C_SP2B, C_G + 128, C_LOC1 + 128,
                           C_LOC2 + 128, C_LOC3)
                # (layout-independent: cols come from the C_* constants)
                rhss = (
                    sv_t[su][:, sp1c : sp1c + 65],
                    sv_t[su][:, sp2c : sp2c + 65],
                    gv_t[su][:],
                    lv_t[su][:, bb * 65 : bb * 65 + 65],
                    lv_t[su][:, (bb + 1) * 65 : (bb + 1) * 65 + 65],
                    lv_t[su][:, (bb + 2) * 65 : (bb + 2) * 65 + 65],
                )
                out = pv[u][:, blk * 128 : blk * 128 + 65]
                for j in range(6):
                    mm = nc.tensor.matmul(
                        out, pp[p % 4][:, lhs[j] : lhs[j] + 128], rhss[j],
                        start=(j == 0), stop=(j == 5),
                    )
                    if blk == 0 and j == 0 and gate_act is not None:
                        mm.wait_op(act, gate_act, GE)  # pp ready
                    if blk == 1 and j == 5 and p >= NP - 3:
                        mm.then_inc(z, 1)  # tail markers for the last copies

        @block.tensor
        def _(tensor):
            tensor.wait_ge(diK[0], 16 * K_TIER[0])
            tensor.wait_ge(mz[0], 16 * 3)  # buffer-0 upper rows zeroed
            emit_scores(0)
            emit_scores(1)
            for p in range(NP):
                s, hb = divmod(p, PPS)
                if p + 2 < NP:
                    s2, hb2 = divmod(p + 2, PPS)
                    if s2 == 0 and hb2 == 2:
                        tensor.wait_ge(diK[1], 16 * K_TIER[1])
                    elif s2 == 0 and hb2 == PRE_P:
                        tensor.wait_ge(diK[2], 16 * K_TIER[2])
                        tensor.wait_ge(mz[1], 16)
                    elif hb2 == 0 and s2 > 0:
                        tensor.wait_ge(diK[s2 + 2], 16 * K_TIER[s2 + 2])
                        if s2 == 1:
                            tensor.wait_ge(mz[2], 16 * 3)  # buffer-1 zeroed
                    emit_scores(p + 2, gate_act=p + 1)
                if s == 0 and hb == 0:
                    tensor.wait_ge(diV[0], 16 * V_TIER[0])
                elif s == 0 and hb == 2:
                    tensor.wait_ge(diV[1], 16 * V_TIER[1])
                elif s == 0 and hb == PRE_P:
                    tensor.wait_ge(diV[2], 16 * V_TIER[2])
                elif hb == 0:
                    tensor.wait_ge(diV[s + 2], 16 * V_TIER[s + 2])
                emit_pv(p, gate_act=p + 1 if p + 2 >= NP else None)

        @block.scalar
        def _(scalar):
            # zero the garbage upper rows of the [64]-loaded K tensors once
            # (their cols are multiplied by qt's zero rows, but leftover SBUF
            # bits could be NaN/inf patterns and NaN*0 = NaN).
            scalar.dma_start(
                lkt_t[0][64:128, 0:PRE_LKT], kz[:, 0:PRE_LKT]
            ).then_inc(mz[0], 16)
            scalar.dma_start(
                skt_t[0][64:128, :], kz[:, 0:SKT_W]
            ).then_inc(mz[0], 16)
            scalar.dma_start(
                gkt_t[0][64:128, :], kz[:, 0:128]
            ).then_inc(mz[0], 16)
            scalar.dma_start(
                lkt_t[0][64:128, PRE_LKT:], kz[:, 0 : LKT_W - PRE_LKT]
            ).then_inc(mz[1], 16)
            for dst, w in ((lkt_t[1], LKT_W), (skt_t[1], SKT_W),
                           (gkt_t[1], 128)):
                scalar.dma_start(
                    dst[64:128, :], kz[:, 0:w]
                ).then_inc(mz[2], 16)
            for p in range(NP):
                u = p % 2
                nc.scalar.activation(
                    pp[p % 4][:, 512:1536], psA[u][:, 0:1024], EXP, scale=0.125
                ).wait_op(zs, p + 1, GE).then_inc(act, 1)

        def emit_schrau(p, gate_zs=None):
            ts = nc.vector.tensor_scalar(
                pp[p % 4][:, 0:512].bitcast(I16),
                psB[p % 2][:, 0:512],
                SCHRAU_MUL, SCHRAU_ADD,
                op0=mybir.AluOpType.mult, op1=mybir.AluOpType.add,
            )
            if gate_zs is not None:
                ts.wait_op(zs, gate_zs, GE)
            ts.then_inc(dact, 1)

        @block.vector
        def _(vector):
            emit_schrau(0, gate_zs=1)
            emit_schrau(1, gate_zs=2)
            for p in range(NP):
                u = p % 2
                u4 = p % 8
                if p + 2 < NP:
                    # schrau(p+2) first, on its own earlier gate, so dact
                    # lands a full period before batch p+4 checks it.
                    emit_schrau(p + 2, gate_zs=p + 3)
                if p >= 8:
                    vector.wait_ge(st, 16 * (p - 7))  # ob[u4] free
                pv3 = pv[u][:].rearrange("q (a c) -> q a c", a=4)
                cp = nc.vector.tensor_copy(ob[u4][:], pv3[:, 0:2, 0:65])
                if p <= NP - 4:
                    cp.wait_op(zs, p + 4, GE)   # scores(p+3) done => PV(p) done
                else:
                    cp.wait_op(z, p - (NP - 4), GE)  # 61->1, 62->2, 63->3
                cp.then_inc(dve, 1)

        @block.gpsimd
        def _(gpsimd):
            for p in range(NP):
                s, hb = divmod(p, PPS)
                b = 2 * hb
                dst = o[s, b * B : (b + 2) * B, :].rearrange(
                    "(blk q) d -> q blk d", blk=2
                )
                gpsimd.dma_start(dst, ob[p % 8][:]).wait_op(
                    dve, p + 1, GE
                ).then_inc(st, 16)

    return nc


def _prepare(inputs):
    import ml_dtypes

    bf = ml_dtypes.bfloat16
    f = np.float32
    q = np.asarray(inputs["query_layer"], f).reshape(NH, T, D)
    k = np.asarray(inputs["key_layer"], f).reshape(NH, T, D)
    v = np.asarray(inputs["value_layer"], f).reshape(NH, T, D)
    sk = np.asarray(inputs["sparse_key"], f).reshape(NH, TSP, D)
    svv = np.asarray(inputs["sparse_value"], f).reshape(NH, TSP, D)
    gk = np.asarray(inputs["global_key"], f).reshape(NH, G, D)
    gvv = np.asarray(inputs["global_value"], f).reshape(NH, G, D)
    am = np.repeat(np.asarray(inputs["attention_mask"], f)[:, 0, 0, :], H, 0)
    sm = np.repeat(np.asarray(inputs["sparse_mask"], f)[:, 0, 0, :], H, 0)
    gm = np.repeat(np.asarray(inputs["global_mask"], f)[:, 0, 0, :], H, 0)

    qt = np.zeros((NH, 128, T), f)
    qt[:, :64] = q.transpose(0, 2, 1)
    qt = qt.astype(bf)

    lkt = np.zeros((NH, 64, LKT_W), f)
    lkt[:, :, B : B + T] = k.transpose(0, 2, 1)
    lkt = lkt.astype(bf)

    skt = np.zeros((NH, 64, SKT_W), f)
    skt[:, :, 160 : 160 + TSP] = sk.transpose(0, 2, 1)
    skt = skt.astype(bf)

    gkt = np.zeros((NH, 64, 128), f)
    gkt[:, :, :G] = gk.transpose(0, 2, 1)
    gkt = gkt.astype(bf)

    # V_aug rows scaled by exp(mask); pad rows are all-zero
    em_l = np.zeros((NH, LKT_W), f)
    em_l[:, B : B + T] = np.exp(am)
    lvp = np.zeros((NH, LKT_W, 65), f)
    lvp[:, B : B + T, :64] = v
    lvp[:, :, 64] = 1.0
    lvp *= em_l[:, :, None]
    lvp = np.ascontiguousarray(
        lvp.reshape(NH, LV_C, 128, 65).transpose(0, 2, 1, 3)
    ).reshape(NH, 128, LV_C * 65).astype(bf)

    SVP_W = 96 + SV_C * 128
    em_s = np.zeros((NH, SVP_W), f)
    em_s[:, 160 : 160 + TSP] = np.exp(sm)
    sv_pad = np.zeros((NH, SVP_W, 65), f)
    sv_pad[:, 160 : 160 + TSP, :64] = svv
    sv_pad[:, :, 64] = 1.0
    sv_pad *= em_s[:, :, None]
    svp = np.empty((NH, 4, 128, SV_C, 65), f)
    for p in range(4):
        svp[:, p] = (
            sv_pad[:, 32 * p : 32 * p + SV_C * 128]
            .reshape(NH, SV_C, 128, 65)
            .transpose(0, 2, 1, 3)
        )
    svp = np.ascontiguousarray(svp.transpose(0, 2, 1, 3, 4)).reshape(
        NH, 128, 4 * SV_C * 65
    ).astype(bf)

    gvp = np.zeros((NH, 128, 65), f)
    gvp[:, :G, :64] = gvv
    gvp[:, :G, 64] = 1.0
    gvp[:, :G] *= np.exp(gm)[:, :, None]
    gvp = gvp.astype(bf)
    kzz = np.zeros((64, LKT_W), bf)

    return [
        {
            "qt": qt[c * SL : (c + 1) * SL],
            "lkt": lkt[c * SL : (c + 1) * SL],
            "skt": skt[c * SL : (c + 1) * SL],
            "gkt": gkt[c * SL : (c + 1) * SL],
            "lv": lvp[c * SL : (c + 1) * SL],
            "sv": svp[c * SL : (c + 1) * SL],
            "gv": gvp[c * SL : (c + 1) * SL],
            "kz": kzz,
        }
        for c in range(NCORES)
    ]


_NC_CACHE = {}
LAST_RESULTS = None


def kernel(**inputs):
    global LAST_RESULTS
    if "nc" not in _NC_CACHE:
        _NC_CACHE["nc"] = _build_bass()
    nc = _NC_CACHE["nc"]
    in_maps = _prepare(inputs)
    res = run_bass_kernel_spmd(nc, in_maps, core_ids=list(range(NCORES)))
    LAST_RESULTS = res
    out = np.empty((NH, T, D), np.float32)
    for c in range(NCORES):
        raw = res.results[c]["o"].astype(np.float32)  # [SL, T, 65]
        out[c * SL : (c + 1) * SL] = raw[:, :, :64] / raw[:, :, 64:65]
    return out.reshape(N, H, T, D)
